# revision 11
# baseline (speedup 1.0000x reference)
"""AngularDescriptor Trainium2 kernel (8 NeuronCores, SPMD + AllReduce).

Per core: T/8 triplets.  Device computes Chebyshev/Legendre bases, the
per-pair-type radial einsum (PE matmul with fixed block-diag weights after a
4-way tj/tk one-hot expansion; 4-way ti select on DVE), the outer product
ang = (g_ij*g_ik) (x) P_l, and segment-sums ang into q[20000,8,4] via
gpsimd.dma_scatter_add.  HW scatter-add loses duplicate indices within one
instruction (last-write-wins race), so the host orders each shard's blocks
into occurrence-rank classes (class r = r-th block of an atom on this core):
within a class all atom indices are unique.  Blocks are SEG=16 same-atom
triplet groups formed on the GLOBAL atom-sorted order and dealt round-robin
to cores, which keeps SEG padding global and class sizes balanced.  Classes
are cut into chunks that rotate over KACC DRAM accumulators, so
same-accumulator scatters serialize (WAW dep) while different-accumulator
scatters overlap.  Padding slots scatter to distinct dummy atom rows
(20000..20479) so one uniform program serves all cores.

The q rows are split into NGRP=4 groups of 32 q-partitions; each group's
K-way add + AllReduce + output DMA runs mid-stream as soon as its scatters
complete, so only the last (quarter-sized) group's reduce sits on the tail.

Layout/pipeline notes:
 - The tj/tk one-hot is FUSED into the Chebyshev recurrence: the recurrence
   state B[h,q,k] = onehot(q)*T_k(x_h)*u_h runs in layout [p, h, q, k, J]
   (J innermost => all DVE ops are long-run stride-1 bf16, 2x mode).  The
   PE transpose reads pair-column c via the strided AP [p, feat(64), jj(2)]
   so the (feat,jj) row interleave costs nothing on DVE.
 - W output columns are (jj, h, ti, d) with d innermost; the ti-select adds
   are contiguous 8-wide slices.  q column order is (l,d); the host
   transposes to (d,l) for free.
 - Legendre P is l-major [p, l, J]; P_0==1 is never materialized (the l=0
   partials pre-reduce h itself).  Pre-reduce trees pair contiguous halves
   so every level keeps >=32-element runs.
 - Per macro the PE work is 8 batches of (8 transposes -> 1 ACT cin copy ->
   8 matmuls -> 1 ACT cnd copy); DVE emits next macro's feature build before
   this macro's post-processing so it never waits on PE/ACT.
"""
import sys

sys.path.insert(0, "/opt/trn_rl_repo")
import numpy as np

from concourse import bass, bacc, mybir, tile
from concourse.bass_utils import run_bass_kernel_spmd

N_TYPES, N_DESC, K_MAX, L_MAX = 4, 8, 8, 4
R_C = 5.0
N_ATOMS = 20000
N_CORES = 8
DL = N_DESC * L_MAX          # 32
QPAD = 20480                 # 128 * 160
STRIDE = 64                  # q row stride in f32 (256B; scatter needs 256B mult)
J = 128                      # field columns per macro-tile
MACRO = 128 * J              # 16384 triplets per macro
KACC = 3                     # rotating DRAM accumulators
SEG = 16                     # triplets pre-reduced per scattered block
MB = MACRO // SEG            # blocks per macro (1024)
CHUNK = 1024                 # max idxs (blocks) per scatter instruction
NGRP = 4                     # staged reduce groups (32 q-partitions each)
GP = 128 // NGRP             # q partitions per group
F32, BF16, I16 = mybir.dt.float32, mybir.dt.bfloat16, mybir.dt.int16
PI = float(np.pi)


def _host_prep(n_atoms, triplet_index, r_ij, r_ik, cos_theta,
               type_i, type_j, type_k, c_table):
    """Global atom-sort -> SEG-blocks -> deal blocks round-robin to cores ->
    per-core occurrence-rank classes (per reduce group) -> uniform layout.

    Block b of a core lives at partition b%128, columns SEG*(b//128)..+SEG-1
    (block-major columns).  Pad slots use r=r_c so fc=0 => ang=0."""
    import ml_dtypes
    T = triplet_index.shape[0]
    atom_all = np.asarray(triplet_index[:, 0], dtype=np.int64)

    # ---- global blocks ----
    order = np.argsort(atom_all, kind="stable")
    sa = atom_all[order]
    first = np.r_[True, sa[1:] != sa[:-1]]
    idxf = np.where(first)[0]
    counts = np.diff(np.r_[idxf, T])
    uatoms = sa[idxf]
    nblk_per_atom = -(-counts // SEG)
    nblk_tot = int(nblk_per_atom.sum())

    blk_atom = np.repeat(uatoms, nblk_per_atom)
    starts = np.r_[0, np.cumsum(counts)[:-1]]
    blk_rank_g = (np.arange(nblk_tot)
                  - np.repeat(np.r_[0, np.cumsum(nblk_per_atom)[:-1]],
                              nblk_per_atom))
    blk_start = np.repeat(starts, nblk_per_atom) + blk_rank_g * SEG
    blk_cnt = np.minimum(
        np.repeat(counts, nblk_per_atom) - blk_rank_g * SEG, SEG)

    # ---- deal blocks to cores (rotating offset per atom) ----
    core_of = (blk_rank_g + np.repeat(uatoms, nblk_per_atom)) % N_CORES
    rank_c = blk_rank_g // N_CORES

    GA = GP * (QPAD // 128)              # atoms per reduce group (5120)
    grp = np.minimum(blk_atom // GA, NGRP - 1)

    # ---- class sizes (uniform across cores) ----
    maxr = int(rank_c.max()) + 1
    cls_cnt = np.zeros((N_CORES, NGRP, maxr), dtype=np.int64)
    np.add.at(cls_cnt, (core_of, grp, rank_c), 1)
    cls_list = []                        # [padded_blk_count, g, r]; fillers g=None
    o = 0
    g_end = []
    for g in range(NGRP):
        for r in range(maxr):
            mx = int(cls_cnt[:, g, r].max())
            if mx == 0:
                continue
            p = -(-mx // 128) * 128
            cls_list.append([p, g, r])
            o += p
        g_end.append(o)
    fill = (-o) % MB
    if fill:
        cls_list.append([fill, None, None])
        o += fill
    TBLK = o
    nmacro = TBLK // MB
    TPAD = TBLK * SEG
    # reduce group g is complete after macro mg[g]-1 (all its chunks before)
    mg = [min(-(-e // MB), nmacro) for e in g_end]

    # chunk table in blocks: (start_blk, len_blk); chunks never cross class
    # or macro boundaries; filler ranges are never scattered
    chunks = []
    o = 0
    for p, g, r in cls_list:
        if g is not None:
            sblk = 0
            while sblk < p:
                cl = min(CHUNK, p - sblk)
                mstart = (o + sblk) // MB
                if (o + sblk + cl - 1) // MB != mstart:
                    cl = (mstart + 1) * MB - (o + sblk)
                chunks.append((o + sblk, cl))
                sblk += cl
        o += p

    # ---- destination block id per global block ----
    cls_off = {}
    o = 0
    for p, g, r in cls_list:
        if g is not None:
            cls_off[(g, r)] = o
        o += p
    fields_src = dict(r_ij=np.asarray(r_ij, np.float32),
                      r_ik=np.asarray(r_ik, np.float32),
                      ct=np.asarray(cos_theta, np.float32),
                      ti=np.asarray(type_i, np.float32),
                      tj=np.asarray(type_j, np.float32),
                      tk=np.asarray(type_k, np.float32))
    G = TPAD // 128
    cores = []
    for c in range(N_CORES):
        m = core_of == c
        b_atom = blk_atom[m]
        b_g = grp[m]
        b_r = rank_c[m]
        b_start = blk_start[m]
        b_cnt = blk_cnt[m]
        key = b_g * maxr + b_r
        ordk = np.lexsort((np.arange(len(key)), key))
        pos = np.empty(len(key), dtype=np.int64)
        kk = key[ordk]
        kfirst = np.r_[True, kk[1:] != kk[:-1]]
        kidx = np.where(kfirst)[0]
        within = np.arange(len(key)) - np.repeat(kidx, np.diff(np.r_[kidx, len(key)]))
        pos[ordk] = within
        dst_blk = np.array([cls_off[(g, r)] for g, r in zip(b_g, b_r)],
                           dtype=np.int64) + pos

        dev = {}
        for n in fields_src:
            fillv = R_C if n in ("r_ij", "r_ik") else 0.0
            dt = ml_dtypes.bfloat16 if n in ("ti", "tj", "tk") else np.float32
            dev[n] = np.full((128, G), fillv, dtype=dt)
        bidx = np.empty(TBLK, dtype=np.int16)
        bidx[:] = (20000 + (np.arange(TBLK, dtype=np.int64) % 480)).astype(np.int16)
        bidx[dst_blk] = b_atom.astype(np.int16)

        slot_b = np.repeat(dst_blk, b_cnt)
        slot_s = (np.arange(int(b_cnt.sum()))
                  - np.repeat(np.r_[0, np.cumsum(b_cnt)[:-1]], b_cnt))
        src_idx = order[np.repeat(b_start, b_cnt) + slot_s]
        dst_p = slot_b % 128
        dst_c = SEG * (slot_b // 128) + slot_s
        for n in dev:
            dev[n][dst_p, dst_c] = fields_src[n][src_idx]
        arrays = {n: dev[n] for n in dev}
        arrays["idx"] = np.tile(bidx.reshape(TBLK // 16, 16).T, (8, 1)).copy()
        cores.append(arrays)

    # ---- weight table ----
    # basis fold: reference uses (T_k + 1)*u; we feed T_k*u:
    #   c'[d,0] += sum_k c[d,k]
    ctab = np.asarray(c_table, dtype=np.float64).copy()
    ctab[:, :, :, 0] += ctab.sum(axis=3)
    ctab = ctab.astype(np.float32)
    # rows r = jj*64 + f with f = (h,q,k) = h*32+q*8+k  (jj-major: the two
    # half-transposes land jj=0 on PSUM rows 0-63 and jj=1 on rows 64-127)
    # cols o = jj*64 + h*32 + ti*8 + d   (d innermost)
    W4p = np.zeros((128, 128), dtype=np.float32)
    for h in range(2):
        for q in range(4):
            for k in range(8):
                f = h * 32 + q * 8 + k
                for jj in range(2):
                    for d in range(8):
                        for ti in range(4):
                            W4p[jj * 64 + f,
                                jj * 64 + h * 32 + ti * 8 + d] = ctab[ti, q, d, k]
    iotaJ = np.tile(np.arange(4, dtype=np.float32)[:, None], (1, J))
    iotaJ = np.tile(iotaJ.reshape(1, 4 * J), (128, 1))   # [128, 4*J] value=q
    consts = dict(w4=W4p, ident=np.eye(128, dtype=np.float32), iotaj=iotaJ)
    return cores, consts, chunks, nmacro, TPAD, tuple(mg)


def _build(chunks, nmacro, TPAD, mg):
    G = TPAD // 128
    nc = bacc.Bacc(None, target_bir_lowering=False, num_devices=N_CORES,
                   dynamic_dma_scratch_size=32768, num_swdge_queues=1)
    P = {}
    for n in ("r_ij", "r_ik", "ct", "ti", "tj", "tk"):
        fdt = BF16 if n in ("ti", "tj", "tk") else F32
        P[n] = nc.declare_dram_parameter(n, [128, G], fdt, isOutput=False)
    P["idx"] = nc.declare_dram_parameter("idx", [128, TPAD // SEG // 16], I16,
                                         isOutput=False)
    P["w4"] = nc.declare_dram_parameter("w4", [128, 128], F32, isOutput=False)
    P["ident"] = nc.declare_dram_parameter("ident", [128, 128], F32, isOutput=False)
    P["iotaj"] = nc.declare_dram_parameter("iotaj", [128, 4 * J], F32,
                                           isOutput=False)
    out_d = nc.declare_dram_parameter("out", [N_ATOMS, DL], F32, isOutput=True)

    qacc = [nc.dram_tensor(f"qacc{k}", [QPAD, STRIDE], F32) for k in range(KACC)]
    bounce_in = nc.dram_tensor("bounce_in", [128, QPAD * DL // 128], F32)
    bounce_out = nc.dram_tensor("bounce_out", [128, QPAD * DL // 128], F32,
                                addr_space="Shared")

    AF = mybir.ActivationFunctionType
    OP = mybir.AluOpType

    with tile.TileContext(nc) as tc:
        with tc.tile_pool(name="const", bufs=1) as cst:
            w4 = cst.tile([128, 128], BF16)
            ident = cst.tile([128, 128], BF16)
            iotaj = cst.tile([128, 4, J], BF16)
            tmpf = cst.tile([128, 4 * J], F32)
            zero = cst.tile([128, 512], F32)
            halfpi = cst.tile([128, 1], F32)
            nc.vector.memset(halfpi[:], PI / 2)
            negone = cst.tile([128, 1], F32)
            nc.vector.memset(negone[:], -1.0)
            nc.sync.dma_start(out=tmpf[:, :128], in_=P["w4"][:])
            nc.vector.tensor_copy(out=w4[:], in_=tmpf[:, :128])
            nc.sync.dma_start(out=tmpf[:, :128], in_=P["ident"][:])
            nc.vector.tensor_copy(out=ident[:], in_=tmpf[:, :128])
            nc.sync.dma_start(out=tmpf[:], in_=P["iotaj"][:])
            nc.vector.tensor_copy(
                out=iotaj[:].rearrange("p q j -> p (q j)"), in_=tmpf[:])
            nc.vector.memset(zero[:], 0.0)
            qf0 = qacc[0].ap().rearrange("(p r) s -> p (r s)", p=128)
            w0 = QPAD * STRIDE // 128
            for i in range(0, w0, 512):
                nc.sync.dma_start(out=qf0[:, i:i + 512],
                                  in_=zero[:, :min(512, w0 - i)])

            with (
                tc.tile_pool(name="fields", bufs=4) as fpool,
                tc.tile_pool(name="idxp", bufs=4) as ipool,
                tc.tile_pool(name="work", bufs=2) as wpool,
                tc.tile_pool(name="f1p", bufs=2) as f1pool,
                tc.tile_pool(name="cinp", bufs=3) as cpool,
                tc.tile_pool(name="cndp", bufs=2) as cndpool,
                tc.tile_pool(name="postp", bufs=1) as ppool,
                tc.tile_pool(name="redp", bufs=2) as redp,
                tc.tile_pool(name="scat", bufs=3) as spool,
                tc.tile_pool(name="ps1", bufs=2, space="PSUM") as ppool1,
                tc.tile_pool(name="ps2", bufs=2, space="PSUM") as ppool2,
            ):
                by_macro = [[] for _ in range(nmacro)]
                for ci, (s, pl) in enumerate(chunks):
                    by_macro[s // MB].append((ci, s % MB, pl))
                NIC = MB // 16   # idx cols per macro

                NR = QPAD // 128
                qv32 = [q.ap().rearrange("(p r) s -> p r s", p=128)[:, :, :DL]
                        for q in qacc]

                def reduce_group(g):
                    """K-way add + AllReduce + output DMA for q partitions
                    [g*GP, (g+1)*GP)."""
                    p0, p1 = g * GP, (g + 1) * GP
                    acc = redp.tile([128, NR, DL], F32, name="acc")
                    nc.sync.dma_start(out=acc[p0:p1], in_=qv32[0][p0:p1])
                    for k in range(1, KACC):
                        nc.gpsimd.dma_start(out=acc[p0:p1], in_=qv32[k][p0:p1],
                                            accum_op=OP.add)
                    nc.sync.dma_start(out=bounce_in.ap()[p0:p1, :],
                                      in_=acc[p0:p1]
                                      .rearrange("p r s -> p (r s)"))
                    nc.gpsimd.collective_compute(
                        "AllReduce", OP.add,
                        replica_groups=[list(range(N_CORES))],
                        ins=[bounce_in.ap()[p0:p1, :].opt()],
                        outs=[bounce_out.ap()[p0:p1, :].opt()])
                    a0 = p0 * NR
                    a1 = min(p1 * NR, N_ATOMS)
                    if a1 > a0:
                        nc.sync.dma_start(
                            out=out_d.ap().rearrange("a c -> (a c)")
                            [a0 * DL:a1 * DL],
                            in_=bounce_out.ap().rearrange("p f -> (p f)")
                            [a0 * DL:a1 * DL])

                def dma_loads(m):
                    fld = {}
                    for n in ("r_ij", "r_ik", "ct", "ti", "tj", "tk"):
                        fdt = BF16 if n in ("ti", "tj", "tk") else F32
                        t = fpool.tile([128, J], fdt, name=f"fld_{n}")
                        nc.sync.dma_start(out=t[:], in_=P[n][:, m * J:(m + 1) * J])
                        fld[n] = t
                    idxs = ipool.tile([128, NIC], I16, name="idxs")
                    nc.sync.dma_start(out=idxs[:],
                                      in_=P["idx"][:, m * NIC:(m + 1) * NIC])
                    return fld, idxs

                def act_pre(fld):
                    """u = 0.5*sin^2(pi/2 - pi*r/(2rc)); s = (r/rc - 1)^2."""
                    u_both = wpool.tile([128, 2, J], BF16, name="u_both")
                    s_both = wpool.tile([128, 2, J], F32, name="s_both")
                    for half, rn in enumerate(("r_ij", "r_ik")):
                        r = fld[rn]
                        utmp = wpool.tile([128, J], F32, name=f"utmp{half}")
                        nc.scalar.activation(utmp[:], r[:], AF.Sin,
                                             bias=halfpi[:], scale=-PI / (2 * R_C))
                        nc.scalar.activation(u_both[:, half, :], utmp[:], AF.Square,
                                             scale=float(np.sqrt(0.5)))
                        nc.scalar.activation(s_both[:, half, :], r[:], AF.Square,
                                             bias=negone[:], scale=1.0 / R_C)
                    ct2 = wpool.tile([128, J], F32, name="ct2")
                    nc.scalar.activation(ct2[:], fld["ct"][:], AF.Square)
                    return u_both, s_both, ct2

                def build_features(fld, u_both, s_both):
                    """B[h,q,k] = oh(q)*T_k(x_h)*u_h, layout [p, h, q, k, J]."""
                    B = f1pool.tile([128, 2, 4, 8, J], BF16, name="B")
                    x_both = wpool.tile([128, 2, J], BF16, name="x_both")
                    x2_both = wpool.tile([128, 2, J], BF16, name="x2_both")
                    nc.vector.tensor_scalar(out=x_both[:], in0=s_both[:],
                                            scalar1=2.0, scalar2=-1.0,
                                            op0=OP.mult, op1=OP.add)
                    nc.vector.tensor_scalar(out=x2_both[:], in0=s_both[:],
                                            scalar1=4.0, scalar2=-2.0,
                                            op0=OP.mult, op1=OP.add)
                    t2 = wpool.tile([128, 2, J], BF16, name="t2")
                    nc.vector.tensor_copy(out=t2[:, 0, :], in_=fld["tj"][:])
                    nc.vector.tensor_copy(out=t2[:, 1, :], in_=fld["tk"][:])
                    oh = wpool.tile([128, 2, 4, J], BF16, name="oh")
                    nc.vector.tensor_tensor(
                        out=oh[:],
                        in0=t2[:].unsqueeze(2).broadcast_to([128, 2, 4, J]),
                        in1=iotaj[:].unsqueeze(1).broadcast_to([128, 2, 4, J]),
                        op=OP.is_equal)
                    ub = u_both[:].unsqueeze(2).broadcast_to([128, 2, 4, J])
                    xb = x_both[:].unsqueeze(2).broadcast_to([128, 2, 4, J])
                    x2b = x2_both[:].unsqueeze(2).broadcast_to([128, 2, 4, J])
                    nc.vector.tensor_tensor(out=B[:, :, :, 0, :], in0=oh[:],
                                            in1=ub, op=OP.mult)
                    nc.vector.tensor_tensor(out=B[:, :, :, 1, :],
                                            in0=B[:, :, :, 0, :],
                                            in1=xb, op=OP.mult)
                    for k in range(2, 8):
                        nc.vector.tensor_tensor(out=B[:, :, :, k, :],
                                                in0=B[:, :, :, k - 1, :],
                                                in1=x2b, op=OP.mult)
                        nc.vector.tensor_tensor(out=B[:, :, :, k, :],
                                                in0=B[:, :, :, k, :],
                                                in1=B[:, :, :, k - 2, :],
                                                op=OP.subtract)
                    return B

                def pe_stage(B):
                    """8 batches of (8 transposes -> cin copy -> 8 matmuls ->
                    cnd copy).  Transpose input for pair-col c is the strided
                    AP [p, feat(64), jj(2)] -> loaded rows r = 2f+jj."""
                    Bv = B[:].rearrange("p h q k j -> p (h q k) j")
                    cndb = cndpool.tile([128, 64, 128], BF16, name="cndb")
                    for b in range(8):
                        ps1 = ppool1.tile([128, 8, 128], BF16, space="PSUM",
                                          name="ps1")
                        for jcol in range(8):
                            c = 8 * b + jcol
                            nc.tensor.transpose(
                                out=ps1[0:64, jcol, :],
                                in_=Bv[:, :, 2 * c],
                                identity=ident[:],
                                tile_position=(0, 0))
                            nc.tensor.transpose(
                                out=ps1[64:128, jcol, :],
                                in_=Bv[:, :, 2 * c + 1],
                                identity=ident[:],
                                tile_position=(0, 64))
                        cin = cpool.tile([128, 8, 128], BF16, name="cin")
                        nc.scalar.activation(
                            cin[:].rearrange("p c f -> p (c f)"),
                            ps1[:].rearrange("p c f -> p (c f)"), AF.Identity)
                        ps2 = ppool2.tile([128, 8, 128], F32, space="PSUM",
                                          name="ps2")
                        for jcol in range(8):
                            nc.tensor.matmul(out=ps2[:, jcol, :],
                                             lhsT=cin[:, jcol, :],
                                             rhs=w4[:], start=True, stop=True)
                        nc.scalar.activation(
                            cndb[:, 8 * b:8 * b + 8, :]
                            .rearrange("p c f -> p (c f)"),
                            ps2[:].rearrange("p c f -> p (c f)"), AF.Identity)
                    return cndb

                def post_stage(cndb, fld, ct2):
                    """ti-select + product + P_l outer + SEG pre-reduce."""
                    oh_ti = wpool.tile([128, J, 4], BF16, name="oh_ti")
                    nc.vector.tensor_tensor(
                        out=oh_ti[:],
                        in0=fld["ti"][:].unsqueeze(2).broadcast_to([128, J, 4]),
                        in1=iotaj[:, :, 0].unsqueeze(1)
                            .broadcast_to([128, J, 4]),
                        op=OP.is_equal)
                    # cnd cols o = jj*64 + h*32 + ti*8 + d; rows = pair-col.
                    cv = cndb[:].rearrange("p c (jj h f) -> p (c jj) h f",
                                           jj=2, h=2)
                    sel0 = ppool.tile([128, J, 32], BF16, name="sel0")
                    nc.vector.tensor_tensor(
                        out=sel0[:].rearrange("p j (t d) -> p j t d", t=4),
                        in0=cv[:, :, 0, :].rearrange("p j (t d) -> p j t d", t=4),
                        in1=oh_ti[:].unsqueeze(3).broadcast_to([128, J, 4, 8]),
                        op=OP.mult)
                    prod = ppool.tile([128, J, 32], BF16, name="prod")
                    nc.vector.tensor_tensor(out=prod[:], in0=sel0[:],
                                            in1=cv[:, :, 1, :], op=OP.mult)
                    # sum over ti: contiguous 8-wide slices
                    a1 = ppool.tile([128, J, 8], BF16, name="a1")
                    nc.vector.tensor_tensor(out=a1[:], in0=prod[:, :, 0:8],
                                            in1=prod[:, :, 8:16], op=OP.add)
                    a2 = ppool.tile([128, J, 8], BF16, name="a2")
                    nc.vector.tensor_tensor(out=a2[:], in0=prod[:, :, 16:24],
                                            in1=prod[:, :, 24:32], op=OP.add)
                    h8 = ppool.tile([128, J, 8], BF16, name="h8")
                    nc.vector.tensor_tensor(out=h8[:], in0=a1[:],
                                            in1=a2[:], op=OP.add)
                    # Legendre P l-major [p, 3, J]
                    Pt = wpool.tile([128, 3, J], BF16, name="Pt")
                    nc.vector.tensor_copy(out=Pt[:, 0, :], in_=fld["ct"][:])
                    nc.vector.tensor_scalar(out=Pt[:, 1, :], in0=ct2[:],
                                            scalar1=1.5, scalar2=-0.5,
                                            op0=OP.mult, op1=OP.add)
                    p3t = wpool.tile([128, J], BF16, name="p3t")
                    nc.vector.tensor_scalar(out=p3t[:], in0=ct2[:],
                                            scalar1=2.5, scalar2=-1.5,
                                            op0=OP.mult, op1=OP.add)
                    nc.vector.tensor_tensor(out=Pt[:, 2, :], in0=p3t[:],
                                            in1=Pt[:, 0, :], op=OP.mult)
                    # ang for l=1..3 [p, J, 3, 8]
                    ang = ppool.tile([128, J, 3, 8], BF16, name="ang")
                    nc.vector.tensor_tensor(
                        out=ang[:],
                        in0=h8[:].unsqueeze(2).broadcast_to([128, J, 3, 8]),
                        in1=Pt[:].rearrange("p l j -> p j l").unsqueeze(3)
                            .broadcast_to([128, J, 3, 8]),
                        op=OP.mult)
                    # SEG=16 pre-reduce by contiguous halves;
                    # partials [p, MB/128, (l,d)=32] f32, l=0 from h8.
                    partials = spool.tile([128, MB // 128, 32], F32,
                                          name="partials")
                    NB = MB // 128      # blocks per partition per macro (8)
                    hv = h8[:].rearrange("p (g s) d -> p g s d", g=NB)
                    t1 = ppool.tile([128, NB, 8, 8], BF16, name="t1")
                    nc.vector.tensor_tensor(out=t1[:], in0=hv[:, :, 0:8, :],
                                            in1=hv[:, :, 8:16, :], op=OP.add)
                    t2s = ppool.tile([128, NB, 4, 8], BF16, name="t2s")
                    nc.vector.tensor_tensor(out=t2s[:], in0=t1[:, :, 0:4, :],
                                            in1=t1[:, :, 4:8, :], op=OP.add)
                    t3 = ppool.tile([128, NB, 2, 8], BF16, name="t3")
                    nc.vector.tensor_tensor(out=t3[:], in0=t2s[:, :, 0:2, :],
                                            in1=t2s[:, :, 2:4, :], op=OP.add)
                    nc.vector.tensor_tensor(out=partials[:, :, 0:8],
                                            in0=t3[:, :, 0, :],
                                            in1=t3[:, :, 1, :], op=OP.add)
                    av = ang[:].rearrange("p (g s) l d -> p g s (l d)", g=NB)
                    u1 = ppool.tile([128, NB, 8, 24], BF16, name="u1")
                    nc.vector.tensor_tensor(out=u1[:], in0=av[:, :, 0:8, :],
                                            in1=av[:, :, 8:16, :], op=OP.add)
                    u2 = ppool.tile([128, NB, 4, 24], BF16, name="u2")
                    nc.vector.tensor_tensor(out=u2[:], in0=u1[:, :, 0:4, :],
                                            in1=u1[:, :, 4:8, :], op=OP.add)
                    u3 = ppool.tile([128, NB, 2, 24], BF16, name="u3")
                    nc.vector.tensor_tensor(out=u3[:], in0=u2[:, :, 0:2, :],
                                            in1=u2[:, :, 2:4, :], op=OP.add)
                    nc.vector.tensor_tensor(out=partials[:, :, 8:32],
                                            in0=u3[:, :, 0, :],
                                            in1=u3[:, :, 1, :], op=OP.add)
                    return partials

                ld = {0: dma_loads(0)}
                if nmacro > 1:
                    ld[1] = dma_loads(1)
                ap = {0: act_pre(ld[0][0])}
                F1cur = build_features(ld[0][0], ap[0][0], ap[0][1])
                for k in range(1, KACC):
                    qf = qacc[k].ap().rearrange("(p r) s -> p (r s)", p=128)
                    w = QPAD * STRIDE // 128
                    for i in range(0, w, 512):
                        nc.sync.dma_start(out=qf[:, i:i + 512],
                                          in_=zero[:, :min(512, w - i)])

                # reduce group g: emitted after macro mg[g] (one extra macro
                # of slack so its scatter drains are done before the load)
                red_at = {}
                for g in range(NGRP):
                    mm = min(mg[g] + 1, nmacro - 1) if g < NGRP - 1 else nmacro
                    red_at.setdefault(mm, []).append(g)

                for m in range(nmacro):
                    fld, idxs = ld[m]
                    u_both, s_both, ct2 = ap[m]
                    F1use = F1cur
                    if m + 2 < nmacro:
                        ld[m + 2] = dma_loads(m + 2)
                    if m + 1 < nmacro:
                        ap[m + 1] = act_pre(ld[m + 1][0])
                        F1cur = build_features(ld[m + 1][0], ap[m + 1][0],
                                               ap[m + 1][1])
                    cndb = pe_stage(F1use)
                    partials = post_stage(cndb, fld, ct2)
                    ld.pop(m, None)
                    ap.pop(m, None)

                    for (ci, off, pl) in by_macro[m]:
                        nc.gpsimd.dma_scatter_add(
                            qacc[ci % KACC].ap()[:, :DL],
                            partials[:, off // 128:(off + pl) // 128, :],
                            idxs[:, off // 16:(off + pl) // 16],
                            pl, pl, DL, elem_step=STRIDE,
                            queue_num=0)

                    for g in red_at.get(m, []):
                        reduce_group(g)

                for g in red_at.get(nmacro, []):
                    reduce_group(g)
    nc.compile()
    return nc


def _install_ntff_hook():
    """Provide antenv.axon_hooks (missing in this image) via sys.modules so
    run_bass_kernel_spmd(trace=True) can capture NTFF profiles."""
    import types, ctypes, contextlib
    try:
        from antenv.axon_hooks import get_axon_ntff_profile_hook  # noqa: F401
        return
    except ImportError:
        pass
    so_path = "/opt/axon/libaxon_pjrt.so"
    try:
        lib = ctypes.CDLL(so_path)
    except OSError:
        return
    if not hasattr(lib, "axon_start_nrt_profile"):
        return
    lib.axon_start_nrt_profile.argtypes = [ctypes.POINTER(ctypes.c_int64),
                                           ctypes.c_size_t]
    lib.axon_start_nrt_profile.restype = ctypes.c_int64
    lib.axon_stop_nrt_profile.argtypes = [ctypes.c_char_p]
    lib.axon_stop_nrt_profile.restype = ctypes.c_int64

    @contextlib.contextmanager
    def _hook(output_dir, device_ids):
        import jax
        jax.devices()
        if device_ids:
            ids = (ctypes.c_int64 * len(device_ids))(*device_ids)
            rc = lib.axon_start_nrt_profile(ids, len(device_ids))
        else:
            rc = lib.axon_start_nrt_profile(None, 0)
        if rc != 0:
            raise RuntimeError(f"axon_start_nrt_profile rc={rc}")
        try:
            yield
        finally:
            n = lib.axon_stop_nrt_profile(str(output_dir).encode())
            if n <= 0:
                print(f"ntff capture wrote {n} files", flush=True)

    mod = types.ModuleType("antenv.axon_hooks")
    mod.get_axon_ntff_profile_hook = lambda: _hook
    mod.set_axon_ntff_profile_hook = lambda h: None
    import antenv
    sys.modules["antenv.axon_hooks"] = mod
    antenv.axon_hooks = mod


_CACHE = {}


def kernel(n_atoms, triplet_index, r_ij, r_ik, cos_theta,
           type_i, type_j, type_k, c_table, _sim=False, _trace=False):
    cores, consts, chunks, nmacro, TPAD, mg = _host_prep(
        n_atoms, triplet_index, r_ij, r_ik, cos_theta,
        type_i, type_j, type_k, c_table)
    key = (nmacro, TPAD, mg, tuple(chunks))
    if key not in _CACHE:
        _CACHE[key] = _build(chunks, nmacro, TPAD, mg)
    nc = _CACHE[key]
    in_maps = []
    for c in range(N_CORES):
        m = dict(cores[c])
        m.update(consts)
        in_maps.append(m)
    if _sim:
        from concourse import bass_interp
        sim = bass_interp.MultiCoreSim(nc, N_CORES)
        for c in range(N_CORES):
            for k, v in in_maps[c].items():
                sim.cores[c].tensor(k)[:] = v
        sim.simulate()
        out = np.array(sim.cores[0].mem_tensor("out"))
    else:
        if _trace:
            _install_ntff_hook()
        last_err = None
        for _try in range(3):
            try:
                res = run_bass_kernel_spmd(nc, in_maps,
                                           core_ids=list(range(N_CORES)),
                                           trace=_trace)
                out = np.asarray(res.results[0]["out"])
                break
            except Exception as e:  # transient device-unrecoverable after a crash
                last_err = e
        else:
            raise last_err
        kernel.last_exec_ns = res.exec_time_ns
        kernel.last_results = res
    # device q columns are (l, d); reference wants (d, l)
    return (out.reshape(N_ATOMS, L_MAX, N_DESC).transpose(0, 2, 1)
            .astype(np.float32))


# revision 12
# speedup vs baseline: 1.0394x; 1.0394x over previous
"""AngularDescriptor Trainium2 kernel (8 NeuronCores, SPMD + AllReduce).

Per core: T/8 triplets.  Device computes Chebyshev/Legendre bases, the
per-pair-type radial einsum (PE matmul with fixed block-diag weights after a
4-way tj/tk one-hot expansion; 4-way ti select on DVE), the outer product
ang = (g_ij*g_ik) (x) P_l, and segment-sums ang into q[20000,8,4] via
gpsimd.dma_scatter_add.  HW scatter-add loses duplicate indices within one
instruction (last-write-wins race), so the host orders each shard's blocks
into occurrence-rank classes (class r = r-th block of an atom on this core):
within a class all atom indices are unique.  Blocks are SEG=16 same-atom
triplet groups formed on the GLOBAL atom-sorted order and dealt round-robin
to cores, which keeps SEG padding global and class sizes balanced.  Classes
are cut into chunks that rotate over KACC DRAM accumulators, so
same-accumulator scatters serialize (WAW dep) while different-accumulator
scatters overlap.  Padding slots scatter to distinct dummy atom rows
(20000..20479) so one uniform program serves all cores.

The q rows are split into NGRP=4 groups of 32 q-partitions; each group's
K-way add + AllReduce + output DMA runs mid-stream as soon as its scatters
complete, so only the last (quarter-sized) group's reduce sits on the tail.

Layout/pipeline notes:
 - The tj/tk one-hot is FUSED into the Chebyshev recurrence: the recurrence
   state B[h,q,k] = onehot(q)*T_k(x_h)*u_h runs in layout [p, h, q, k, J]
   (J innermost => all DVE ops are long-run stride-1 bf16, 2x mode).  The
   PE transpose reads pair-column c via the strided AP [p, feat(64), jj(2)]
   so the (feat,jj) row interleave costs nothing on DVE.
 - W output columns are (jj, h, ti, d) with d innermost; the ti-select adds
   are contiguous 8-wide slices.  q column order is (l,d); the host
   transposes to (d,l) for free.
 - Legendre P is l-major [p, l, J]; P_0==1 is never materialized (the l=0
   partials pre-reduce h itself).  Pre-reduce trees pair contiguous halves
   so every level keeps >=32-element runs.
 - Per macro the PE work is 8 batches of (8 transposes -> 1 ACT cin copy ->
   8 matmuls -> 1 ACT cnd copy); DVE emits next macro's feature build before
   this macro's post-processing so it never waits on PE/ACT.
"""
import sys

sys.path.insert(0, "/opt/trn_rl_repo")
import numpy as np

from concourse import bass, bacc, mybir, tile
from concourse.bass_utils import run_bass_kernel_spmd

N_TYPES, N_DESC, K_MAX, L_MAX = 4, 8, 8, 4
R_C = 5.0
N_ATOMS = 20000
N_CORES = 8
DL = N_DESC * L_MAX          # 32
QPAD = 20480                 # 128 * 160
STRIDE = 64                  # q row stride in f32 (256B; scatter needs 256B mult)
J = 128                      # field columns per macro-tile
MACRO = 128 * J              # 16384 triplets per macro
KACC = 3                     # rotating DRAM accumulators
SEG = 16                     # triplets pre-reduced per scattered block
MB = MACRO // SEG            # blocks per macro (1024)
CHUNK = 1024                 # max idxs (blocks) per scatter instruction
NGRP = 4                     # staged reduce groups (32 q-partitions each)
GP = 128 // NGRP             # q partitions per group
F32, BF16, I16 = mybir.dt.float32, mybir.dt.bfloat16, mybir.dt.int16
PI = float(np.pi)


def _host_prep(n_atoms, triplet_index, r_ij, r_ik, cos_theta,
               type_i, type_j, type_k, c_table):
    """Global atom-sort -> SEG-blocks -> deal blocks round-robin to cores ->
    per-core occurrence-rank classes (per reduce group) -> uniform layout.

    Block b of a core lives at partition b%128, columns SEG*(b//128)..+SEG-1
    (block-major columns).  Pad slots use r=r_c so fc=0 => ang=0."""
    import ml_dtypes
    T = triplet_index.shape[0]
    atom_all = np.asarray(triplet_index[:, 0], dtype=np.int64)

    # ---- global blocks ----
    order = np.argsort(atom_all, kind="stable")
    sa = atom_all[order]
    first = np.r_[True, sa[1:] != sa[:-1]]
    idxf = np.where(first)[0]
    counts = np.diff(np.r_[idxf, T])
    uatoms = sa[idxf]
    nblk_per_atom = -(-counts // SEG)
    nblk_tot = int(nblk_per_atom.sum())

    blk_atom = np.repeat(uatoms, nblk_per_atom)
    starts = np.r_[0, np.cumsum(counts)[:-1]]
    blk_rank_g = (np.arange(nblk_tot)
                  - np.repeat(np.r_[0, np.cumsum(nblk_per_atom)[:-1]],
                              nblk_per_atom))
    blk_start = np.repeat(starts, nblk_per_atom) + blk_rank_g * SEG
    blk_cnt = np.minimum(
        np.repeat(counts, nblk_per_atom) - blk_rank_g * SEG, SEG)

    # ---- deal blocks to cores (rotating offset per atom) ----
    core_of = (blk_rank_g + np.repeat(uatoms, nblk_per_atom)) % N_CORES
    rank_c = blk_rank_g // N_CORES

    GA = GP * (QPAD // 128)              # atoms per reduce group (5120)
    grp = np.minimum(blk_atom // GA, NGRP - 1)

    # ---- class sizes (uniform across cores) ----
    maxr = int(rank_c.max()) + 1
    cls_cnt = np.zeros((N_CORES, NGRP, maxr), dtype=np.int64)
    np.add.at(cls_cnt, (core_of, grp, rank_c), 1)
    cls_list = []                        # [padded_blk_count, g, r]; fillers g=None
    o = 0
    g_end = []
    for g in range(NGRP):
        for r in range(maxr):
            mx = int(cls_cnt[:, g, r].max())
            if mx == 0:
                continue
            p = -(-mx // 128) * 128
            cls_list.append([p, g, r])
            o += p
        g_end.append(o)
    fill = (-o) % MB
    if fill:
        cls_list.append([fill, None, None])
        o += fill
    TBLK = o
    nmacro = TBLK // MB
    TPAD = TBLK * SEG
    # reduce group g is complete after macro mg[g]-1 (all its chunks before)
    mg = [min(-(-e // MB), nmacro) for e in g_end]

    # chunk table in blocks: (start_blk, len_blk); chunks never cross class
    # or macro boundaries; filler ranges are never scattered
    chunks = []
    o = 0
    for p, g, r in cls_list:
        if g is not None:
            sblk = 0
            while sblk < p:
                cl = min(CHUNK, p - sblk)
                mstart = (o + sblk) // MB
                if (o + sblk + cl - 1) // MB != mstart:
                    cl = (mstart + 1) * MB - (o + sblk)
                chunks.append((o + sblk, cl))
                sblk += cl
        o += p

    # ---- destination block id per global block ----
    cls_off = {}
    o = 0
    for p, g, r in cls_list:
        if g is not None:
            cls_off[(g, r)] = o
        o += p
    fields_src = dict(r_ij=np.asarray(r_ij, np.float32),
                      r_ik=np.asarray(r_ik, np.float32),
                      ct=np.asarray(cos_theta, np.float32),
                      ti=np.asarray(type_i, np.float32),
                      tj=np.asarray(type_j, np.float32),
                      tk=np.asarray(type_k, np.float32))
    G = TPAD // 128
    cores = []
    for c in range(N_CORES):
        m = core_of == c
        b_atom = blk_atom[m]
        b_g = grp[m]
        b_r = rank_c[m]
        b_start = blk_start[m]
        b_cnt = blk_cnt[m]
        key = b_g * maxr + b_r
        ordk = np.lexsort((np.arange(len(key)), key))
        pos = np.empty(len(key), dtype=np.int64)
        kk = key[ordk]
        kfirst = np.r_[True, kk[1:] != kk[:-1]]
        kidx = np.where(kfirst)[0]
        within = np.arange(len(key)) - np.repeat(kidx, np.diff(np.r_[kidx, len(key)]))
        pos[ordk] = within
        dst_blk = np.array([cls_off[(g, r)] for g, r in zip(b_g, b_r)],
                           dtype=np.int64) + pos

        dev = {}
        for n in fields_src:
            fillv = R_C if n in ("r_ij", "r_ik") else 0.0
            dt = ml_dtypes.bfloat16 if n in ("ti", "tj", "tk") else np.float32
            dev[n] = np.full((128, G), fillv, dtype=dt)
        bidx = np.empty(TBLK, dtype=np.int16)
        bidx[:] = (20000 + (np.arange(TBLK, dtype=np.int64) % 480)).astype(np.int16)
        bidx[dst_blk] = b_atom.astype(np.int16)

        slot_b = np.repeat(dst_blk, b_cnt)
        slot_s = (np.arange(int(b_cnt.sum()))
                  - np.repeat(np.r_[0, np.cumsum(b_cnt)[:-1]], b_cnt))
        src_idx = order[np.repeat(b_start, b_cnt) + slot_s]
        dst_p = slot_b % 128
        dst_c = SEG * (slot_b // 128) + slot_s
        for n in dev:
            dev[n][dst_p, dst_c] = fields_src[n][src_idx]
        arrays = {n: dev[n] for n in dev}
        arrays["idx"] = np.tile(bidx.reshape(TBLK // 16, 16).T, (8, 1)).copy()
        cores.append(arrays)

    # ---- weight table ----
    # basis fold: reference uses (T_k + 1)*u; we feed T_k*u:
    #   c'[d,0] += sum_k c[d,k]
    ctab = np.asarray(c_table, dtype=np.float64).copy()
    ctab[:, :, :, 0] += ctab.sum(axis=3)
    ctab = ctab.astype(np.float32)
    # rows r = jj*64 + f with f = (h,q,k) = h*32+q*8+k  (jj-major: the two
    # half-transposes land jj=0 on PSUM rows 0-63 and jj=1 on rows 64-127)
    # cols o = jj*64 + h*32 + d*4 + ti   (ti innermost)
    W4p = np.zeros((128, 128), dtype=np.float32)
    for h in range(2):
        for q in range(4):
            for k in range(8):
                f = h * 32 + q * 8 + k
                for jj in range(2):
                    for d in range(8):
                        for ti in range(4):
                            W4p[jj * 64 + f,
                                jj * 64 + h * 32 + d * 4 + ti] = ctab[ti, q, d, k]
    iotaJ = np.tile(np.arange(4, dtype=np.float32)[:, None], (1, J))
    iotaJ = np.tile(iotaJ.reshape(1, 4 * J), (128, 1))   # [128, 4*J] value=q
    consts = dict(w4=W4p, ident=np.eye(128, dtype=np.float32), iotaj=iotaJ)
    return cores, consts, chunks, nmacro, TPAD, tuple(mg)


def _build(chunks, nmacro, TPAD, mg):
    G = TPAD // 128
    nc = bacc.Bacc(None, target_bir_lowering=False, num_devices=N_CORES,
                   dynamic_dma_scratch_size=32768, num_swdge_queues=1)
    P = {}
    for n in ("r_ij", "r_ik", "ct", "ti", "tj", "tk"):
        fdt = BF16 if n in ("ti", "tj", "tk") else F32
        P[n] = nc.declare_dram_parameter(n, [128, G], fdt, isOutput=False)
    P["idx"] = nc.declare_dram_parameter("idx", [128, TPAD // SEG // 16], I16,
                                         isOutput=False)
    P["w4"] = nc.declare_dram_parameter("w4", [128, 128], F32, isOutput=False)
    P["ident"] = nc.declare_dram_parameter("ident", [128, 128], F32, isOutput=False)
    P["iotaj"] = nc.declare_dram_parameter("iotaj", [128, 4 * J], F32,
                                           isOutput=False)
    out_d = nc.declare_dram_parameter("out", [N_ATOMS, DL], F32, isOutput=True)

    qacc = [nc.dram_tensor(f"qacc{k}", [QPAD, STRIDE], F32) for k in range(KACC)]
    bounce_in = nc.dram_tensor("bounce_in", [128, QPAD * DL // 128], F32)
    bounce_out = nc.dram_tensor("bounce_out", [128, QPAD * DL // 128], F32,
                                addr_space="Shared")

    AF = mybir.ActivationFunctionType
    OP = mybir.AluOpType

    with tile.TileContext(nc) as tc:
        with tc.tile_pool(name="const", bufs=1) as cst:
            w4 = cst.tile([128, 128], BF16)
            ident = cst.tile([128, 128], BF16)
            iotaj = cst.tile([128, 4, J], BF16)
            tmpf = cst.tile([128, 4 * J], F32)
            zero = cst.tile([128, 512], F32)
            halfpi = cst.tile([128, 1], F32)
            nc.vector.memset(halfpi[:], PI / 2)
            negone = cst.tile([128, 1], F32)
            nc.vector.memset(negone[:], -1.0)
            nc.sync.dma_start(out=tmpf[:, :128], in_=P["w4"][:])
            nc.vector.tensor_copy(out=w4[:], in_=tmpf[:, :128])
            nc.sync.dma_start(out=tmpf[:, :128], in_=P["ident"][:])
            nc.vector.tensor_copy(out=ident[:], in_=tmpf[:, :128])
            nc.sync.dma_start(out=tmpf[:], in_=P["iotaj"][:])
            nc.vector.tensor_copy(
                out=iotaj[:].rearrange("p q j -> p (q j)"), in_=tmpf[:])
            nc.vector.memset(zero[:], 0.0)
            qf0 = qacc[0].ap().rearrange("(p r) s -> p (r s)", p=128)
            w0 = QPAD * STRIDE // 128
            for i in range(0, w0, 512):
                nc.sync.dma_start(out=qf0[:, i:i + 512],
                                  in_=zero[:, :min(512, w0 - i)])

            with (
                tc.tile_pool(name="fields", bufs=4) as fpool,
                tc.tile_pool(name="idxp", bufs=4) as ipool,
                tc.tile_pool(name="work", bufs=2) as wpool,
                tc.tile_pool(name="f1p", bufs=2) as f1pool,
                tc.tile_pool(name="cinp", bufs=3) as cpool,
                tc.tile_pool(name="cndp", bufs=2) as cndpool,
                tc.tile_pool(name="postp", bufs=1) as ppool,
                tc.tile_pool(name="redp", bufs=2) as redp,
                tc.tile_pool(name="scat", bufs=3) as spool,
                tc.tile_pool(name="ps1", bufs=2, space="PSUM") as ppool1,
                tc.tile_pool(name="ps2", bufs=2, space="PSUM") as ppool2,
            ):
                by_macro = [[] for _ in range(nmacro)]
                for ci, (s, pl) in enumerate(chunks):
                    by_macro[s // MB].append((ci, s % MB, pl))
                NIC = MB // 16   # idx cols per macro

                NR = QPAD // 128
                qv32 = [q.ap().rearrange("(p r) s -> p r s", p=128)[:, :, :DL]
                        for q in qacc]

                def reduce_group(g):
                    """K-way add + AllReduce + output DMA for q partitions
                    [g*GP, (g+1)*GP)."""
                    p0, p1 = g * GP, (g + 1) * GP
                    acc = redp.tile([128, NR, DL], F32, name="acc")
                    nc.sync.dma_start(out=acc[p0:p1], in_=qv32[0][p0:p1])
                    for k in range(1, KACC):
                        nc.gpsimd.dma_start(out=acc[p0:p1], in_=qv32[k][p0:p1],
                                            accum_op=OP.add)
                    nc.sync.dma_start(out=bounce_in.ap()[p0:p1, :],
                                      in_=acc[p0:p1]
                                      .rearrange("p r s -> p (r s)"))
                    nc.gpsimd.collective_compute(
                        "AllReduce", OP.add,
                        replica_groups=[list(range(N_CORES))],
                        ins=[bounce_in.ap()[p0:p1, :].opt()],
                        outs=[bounce_out.ap()[p0:p1, :].opt()])
                    a0 = p0 * NR
                    a1 = min(p1 * NR, N_ATOMS)
                    if a1 > a0:
                        nc.sync.dma_start(
                            out=out_d.ap().rearrange("a c -> (a c)")
                            [a0 * DL:a1 * DL],
                            in_=bounce_out.ap().rearrange("p f -> (p f)")
                            [a0 * DL:a1 * DL])

                def dma_loads(m):
                    fld = {}
                    for n in ("r_ij", "r_ik", "ct", "ti", "tj", "tk"):
                        fdt = BF16 if n in ("ti", "tj", "tk") else F32
                        t = fpool.tile([128, J], fdt, name=f"fld_{n}")
                        nc.sync.dma_start(out=t[:], in_=P[n][:, m * J:(m + 1) * J])
                        fld[n] = t
                    idxs = ipool.tile([128, NIC], I16, name="idxs")
                    nc.sync.dma_start(out=idxs[:],
                                      in_=P["idx"][:, m * NIC:(m + 1) * NIC])
                    return fld, idxs

                def act_pre(fld):
                    """u = 0.5*sin^2(pi/2 - pi*r/(2rc)); s = (r/rc - 1)^2."""
                    u_both = wpool.tile([128, 2, J], BF16, name="u_both")
                    s_both = wpool.tile([128, 2, J], F32, name="s_both")
                    for half, rn in enumerate(("r_ij", "r_ik")):
                        r = fld[rn]
                        utmp = wpool.tile([128, J], F32, name=f"utmp{half}")
                        nc.scalar.activation(utmp[:], r[:], AF.Sin,
                                             bias=halfpi[:], scale=-PI / (2 * R_C))
                        nc.scalar.activation(u_both[:, half, :], utmp[:], AF.Square,
                                             scale=float(np.sqrt(0.5)))
                        nc.scalar.activation(s_both[:, half, :], r[:], AF.Square,
                                             bias=negone[:], scale=1.0 / R_C)
                    ct2 = wpool.tile([128, J], F32, name="ct2")
                    nc.scalar.activation(ct2[:], fld["ct"][:], AF.Square)
                    return u_both, s_both, ct2

                def build_features(fld, u_both, s_both):
                    """B[h,q,k] = oh(q)*T_k(x_h)*u_h, layout [p, h, q, k, J]."""
                    B = f1pool.tile([128, 2, 4, 8, J], BF16, name="B")
                    x_both = wpool.tile([128, 2, J], BF16, name="x_both")
                    x2_both = wpool.tile([128, 2, J], BF16, name="x2_both")
                    nc.vector.tensor_scalar(out=x_both[:], in0=s_both[:],
                                            scalar1=2.0, scalar2=-1.0,
                                            op0=OP.mult, op1=OP.add)
                    nc.vector.tensor_scalar(out=x2_both[:], in0=s_both[:],
                                            scalar1=4.0, scalar2=-2.0,
                                            op0=OP.mult, op1=OP.add)
                    t2 = wpool.tile([128, 2, J], BF16, name="t2")
                    nc.vector.tensor_copy(out=t2[:, 0, :], in_=fld["tj"][:])
                    nc.vector.tensor_copy(out=t2[:, 1, :], in_=fld["tk"][:])
                    oh = wpool.tile([128, 2, 4, J], BF16, name="oh")
                    nc.vector.tensor_tensor(
                        out=oh[:],
                        in0=t2[:].unsqueeze(2).broadcast_to([128, 2, 4, J]),
                        in1=iotaj[:].unsqueeze(1).broadcast_to([128, 2, 4, J]),
                        op=OP.is_equal)
                    ub = u_both[:].unsqueeze(2).broadcast_to([128, 2, 4, J])
                    xb = x_both[:].unsqueeze(2).broadcast_to([128, 2, 4, J])
                    x2b = x2_both[:].unsqueeze(2).broadcast_to([128, 2, 4, J])
                    nc.vector.tensor_tensor(out=B[:, :, :, 0, :], in0=oh[:],
                                            in1=ub, op=OP.mult)
                    nc.vector.tensor_tensor(out=B[:, :, :, 1, :],
                                            in0=B[:, :, :, 0, :],
                                            in1=xb, op=OP.mult)
                    for k in range(2, 8):
                        nc.vector.tensor_tensor(out=B[:, :, :, k, :],
                                                in0=B[:, :, :, k - 1, :],
                                                in1=x2b, op=OP.mult)
                        nc.vector.tensor_tensor(out=B[:, :, :, k, :],
                                                in0=B[:, :, :, k, :],
                                                in1=B[:, :, :, k - 2, :],
                                                op=OP.subtract)
                    return B

                def pe_stage(B):
                    """8 batches of (8 transposes -> cin copy -> 8 matmuls ->
                    cnd copy).  Transpose input for pair-col c is the strided
                    AP [p, feat(64), jj(2)] -> loaded rows r = 2f+jj."""
                    Bv = B[:].rearrange("p h q k j -> p (h q k) j")
                    cndb = cndpool.tile([128, 64, 128], BF16, name="cndb")

                    def mm_batch(bb, cin):
                        ps2 = ppool2.tile([128, 8, 128], F32, space="PSUM",
                                          name="ps2")
                        for jcol in range(8):
                            nc.tensor.matmul(out=ps2[:, jcol, :],
                                             lhsT=cin[:, jcol, :],
                                             rhs=w4[:], start=True, stop=True)
                        nc.scalar.activation(
                            cndb[:, 8 * bb:8 * bb + 8, :]
                            .rearrange("p c f -> p (c f)"),
                            ps2[:].rearrange("p c f -> p (c f)"), AF.Identity)

                    prev = None
                    for b in range(8):
                        ps1 = ppool1.tile([128, 8, 128], BF16, space="PSUM",
                                          name="ps1")
                        for jcol in range(8):
                            c = 8 * b + jcol
                            nc.tensor.transpose(
                                out=ps1[0:64, jcol, :],
                                in_=Bv[:, :, 2 * c],
                                identity=ident[:],
                                tile_position=(0, 0))
                            nc.tensor.transpose(
                                out=ps1[64:128, jcol, :],
                                in_=Bv[:, :, 2 * c + 1],
                                identity=ident[:],
                                tile_position=(0, 64))
                        cin = cpool.tile([128, 8, 128], BF16, name="cin")
                        nc.scalar.activation(
                            cin[:].rearrange("p c f -> p (c f)"),
                            ps1[:].rearrange("p c f -> p (c f)"), AF.Identity)
                        if prev is not None:
                            mm_batch(prev[0], prev[1])
                        prev = (b, cin)
                    mm_batch(prev[0], prev[1])
                    return cndb

                def post_stage(cndb, fld, ct2):
                    """ti-select + product + P_l outer + SEG pre-reduce."""
                    oh_ti = wpool.tile([128, J, 4], BF16, name="oh_ti")
                    nc.vector.tensor_tensor(
                        out=oh_ti[:],
                        in0=fld["ti"][:].unsqueeze(2).broadcast_to([128, J, 4]),
                        in1=iotaj[:, :, 0].unsqueeze(1)
                            .broadcast_to([128, J, 4]),
                        op=OP.is_equal)
                    # cnd cols o = jj*64 + h*32 + ti*8 + d; rows = pair-col.
                    cv = cndb[:].rearrange("p c (jj h f) -> p (c jj) h f",
                                           jj=2, h=2)
                    sel0 = ppool.tile([128, J, 32], BF16, name="sel0")
                    nc.vector.tensor_tensor(
                        out=sel0[:].rearrange("p j (d t) -> p j d t", t=4),
                        in0=cv[:, :, 0, :].rearrange("p j (d t) -> p j d t", t=4),
                        in1=oh_ti[:].unsqueeze(2).broadcast_to([128, J, 8, 4]),
                        op=OP.mult)
                    prod = ppool.tile([128, J, 32], BF16, name="prod")
                    nc.vector.tensor_tensor(out=prod[:], in0=sel0[:],
                                            in1=cv[:, :, 1, :], op=OP.mult)
                    # sum over ti (innermost 4)
                    pv = prod[:].rearrange("p j (d a b) -> p j d a b", a=2, b=2)
                    a0 = ppool.tile([128, J, 8, 2], BF16, name="a0")
                    nc.vector.tensor_tensor(out=a0[:], in0=pv[:, :, :, 0, :],
                                            in1=pv[:, :, :, 1, :], op=OP.add)
                    h8 = ppool.tile([128, J, 8], BF16, name="h8")
                    nc.vector.tensor_tensor(out=h8[:], in0=a0[:, :, :, 0],
                                            in1=a0[:, :, :, 1], op=OP.add)
                    # Legendre P l-major [p, 3, J]
                    Pt = wpool.tile([128, 3, J], BF16, name="Pt")
                    nc.vector.tensor_copy(out=Pt[:, 0, :], in_=fld["ct"][:])
                    nc.vector.tensor_scalar(out=Pt[:, 1, :], in0=ct2[:],
                                            scalar1=1.5, scalar2=-0.5,
                                            op0=OP.mult, op1=OP.add)
                    p3t = wpool.tile([128, J], BF16, name="p3t")
                    nc.vector.tensor_scalar(out=p3t[:], in0=ct2[:],
                                            scalar1=2.5, scalar2=-1.5,
                                            op0=OP.mult, op1=OP.add)
                    nc.vector.tensor_tensor(out=Pt[:, 2, :], in0=p3t[:],
                                            in1=Pt[:, 0, :], op=OP.mult)
                    # ang for l=1..3 [p, J, 3, 8]
                    ang = ppool.tile([128, J, 3, 8], BF16, name="ang")
                    nc.vector.tensor_tensor(
                        out=ang[:],
                        in0=Pt[:].rearrange("p l j -> p j l").unsqueeze(3)
                            .broadcast_to([128, J, 3, 8]),
                        in1=h8[:].unsqueeze(2).broadcast_to([128, J, 3, 8]),
                        op=OP.mult)
                    # SEG=16 pre-reduce by contiguous halves;
                    # partials [p, MB/128, (l,d)=32] f32, l=0 from h8.
                    partials = spool.tile([128, MB // 128, 32], F32,
                                          name="partials")
                    NB = MB // 128      # blocks per partition per macro (8)
                    hv = h8[:].rearrange("p (g s) d -> p g s d", g=NB)
                    t1 = ppool.tile([128, NB, 8, 8], BF16, name="t1")
                    nc.vector.tensor_tensor(out=t1[:], in0=hv[:, :, 0:8, :],
                                            in1=hv[:, :, 8:16, :], op=OP.add)
                    t2s = ppool.tile([128, NB, 4, 8], BF16, name="t2s")
                    nc.vector.tensor_tensor(out=t2s[:], in0=t1[:, :, 0:4, :],
                                            in1=t1[:, :, 4:8, :], op=OP.add)
                    t3 = ppool.tile([128, NB, 2, 8], BF16, name="t3")
                    nc.vector.tensor_tensor(out=t3[:], in0=t2s[:, :, 0:2, :],
                                            in1=t2s[:, :, 2:4, :], op=OP.add)
                    nc.vector.tensor_tensor(out=partials[:, :, 0:8],
                                            in0=t3[:, :, 0, :],
                                            in1=t3[:, :, 1, :], op=OP.add)
                    av = ang[:].rearrange("p (g s) l d -> p g s (l d)", g=NB)
                    u1 = ppool.tile([128, NB, 8, 24], BF16, name="u1")
                    nc.vector.tensor_tensor(out=u1[:], in0=av[:, :, 0:8, :],
                                            in1=av[:, :, 8:16, :], op=OP.add)
                    u2 = ppool.tile([128, NB, 4, 24], BF16, name="u2")
                    nc.vector.tensor_tensor(out=u2[:], in0=u1[:, :, 0:4, :],
                                            in1=u1[:, :, 4:8, :], op=OP.add)
                    u3 = ppool.tile([128, NB, 2, 24], BF16, name="u3")
                    nc.vector.tensor_tensor(out=u3[:], in0=u2[:, :, 0:2, :],
                                            in1=u2[:, :, 2:4, :], op=OP.add)
                    nc.vector.tensor_tensor(out=partials[:, :, 8:32],
                                            in0=u3[:, :, 0, :],
                                            in1=u3[:, :, 1, :], op=OP.add)
                    return partials

                ld = {0: dma_loads(0)}
                if nmacro > 1:
                    ld[1] = dma_loads(1)
                ap = {0: act_pre(ld[0][0])}
                F1cur = build_features(ld[0][0], ap[0][0], ap[0][1])
                for k in range(1, KACC):
                    qf = qacc[k].ap().rearrange("(p r) s -> p (r s)", p=128)
                    w = QPAD * STRIDE // 128
                    for i in range(0, w, 512):
                        nc.sync.dma_start(out=qf[:, i:i + 512],
                                          in_=zero[:, :min(512, w - i)])

                # reduce group g: emitted after macro mg[g] (one extra macro
                # of slack so its scatter drains are done before the load)
                red_at = {}
                for g in range(NGRP):
                    mm = min(mg[g] + 1, nmacro - 1) if g < NGRP - 1 else nmacro
                    red_at.setdefault(mm, []).append(g)

                for m in range(nmacro):
                    fld, idxs = ld[m]
                    u_both, s_both, ct2 = ap[m]
                    F1use = F1cur
                    if m + 2 < nmacro:
                        ld[m + 2] = dma_loads(m + 2)
                    if m + 1 < nmacro:
                        ap[m + 1] = act_pre(ld[m + 1][0])
                        F1cur = build_features(ld[m + 1][0], ap[m + 1][0],
                                               ap[m + 1][1])
                    cndb = pe_stage(F1use)
                    partials = post_stage(cndb, fld, ct2)
                    ld.pop(m, None)
                    ap.pop(m, None)

                    for (ci, off, pl) in by_macro[m]:
                        nc.gpsimd.dma_scatter_add(
                            qacc[ci % KACC].ap()[:, :DL],
                            partials[:, off // 128:(off + pl) // 128, :],
                            idxs[:, off // 16:(off + pl) // 16],
                            pl, pl, DL, elem_step=STRIDE,
                            queue_num=0)

                    for g in red_at.get(m, []):
                        reduce_group(g)

                for g in red_at.get(nmacro, []):
                    reduce_group(g)
    nc.compile()
    return nc


def _install_ntff_hook():
    """Provide antenv.axon_hooks (missing in this image) via sys.modules so
    run_bass_kernel_spmd(trace=True) can capture NTFF profiles."""
    import types, ctypes, contextlib
    try:
        from antenv.axon_hooks import get_axon_ntff_profile_hook  # noqa: F401
        return
    except ImportError:
        pass
    so_path = "/opt/axon/libaxon_pjrt.so"
    try:
        lib = ctypes.CDLL(so_path)
    except OSError:
        return
    if not hasattr(lib, "axon_start_nrt_profile"):
        return
    lib.axon_start_nrt_profile.argtypes = [ctypes.POINTER(ctypes.c_int64),
                                           ctypes.c_size_t]
    lib.axon_start_nrt_profile.restype = ctypes.c_int64
    lib.axon_stop_nrt_profile.argtypes = [ctypes.c_char_p]
    lib.axon_stop_nrt_profile.restype = ctypes.c_int64

    @contextlib.contextmanager
    def _hook(output_dir, device_ids):
        import jax
        jax.devices()
        if device_ids:
            ids = (ctypes.c_int64 * len(device_ids))(*device_ids)
            rc = lib.axon_start_nrt_profile(ids, len(device_ids))
        else:
            rc = lib.axon_start_nrt_profile(None, 0)
        if rc != 0:
            raise RuntimeError(f"axon_start_nrt_profile rc={rc}")
        try:
            yield
        finally:
            n = lib.axon_stop_nrt_profile(str(output_dir).encode())
            if n <= 0:
                print(f"ntff capture wrote {n} files", flush=True)

    mod = types.ModuleType("antenv.axon_hooks")
    mod.get_axon_ntff_profile_hook = lambda: _hook
    mod.set_axon_ntff_profile_hook = lambda h: None
    import antenv
    sys.modules["antenv.axon_hooks"] = mod
    antenv.axon_hooks = mod


_CACHE = {}


def kernel(n_atoms, triplet_index, r_ij, r_ik, cos_theta,
           type_i, type_j, type_k, c_table, _sim=False, _trace=False):
    cores, consts, chunks, nmacro, TPAD, mg = _host_prep(
        n_atoms, triplet_index, r_ij, r_ik, cos_theta,
        type_i, type_j, type_k, c_table)
    key = (nmacro, TPAD, mg, tuple(chunks))
    if key not in _CACHE:
        _CACHE[key] = _build(chunks, nmacro, TPAD, mg)
    nc = _CACHE[key]
    in_maps = []
    for c in range(N_CORES):
        m = dict(cores[c])
        m.update(consts)
        in_maps.append(m)
    if _sim:
        from concourse import bass_interp
        sim = bass_interp.MultiCoreSim(nc, N_CORES)
        for c in range(N_CORES):
            for k, v in in_maps[c].items():
                sim.cores[c].tensor(k)[:] = v
        sim.simulate()
        out = np.array(sim.cores[0].mem_tensor("out"))
    else:
        if _trace:
            _install_ntff_hook()
        last_err = None
        for _try in range(3):
            try:
                res = run_bass_kernel_spmd(nc, in_maps,
                                           core_ids=list(range(N_CORES)),
                                           trace=_trace)
                out = np.asarray(res.results[0]["out"])
                break
            except Exception as e:  # transient device-unrecoverable after a crash
                last_err = e
        else:
            raise last_err
        kernel.last_exec_ns = res.exec_time_ns
        kernel.last_results = res
    # device q columns are (l, d); reference wants (d, l)
    return (out.reshape(N_ATOMS, L_MAX, N_DESC).transpose(0, 2, 1)
            .astype(np.float32))


# revision 13
# speedup vs baseline: 1.7215x; 1.6563x over previous
"""AngularDescriptor Trainium2 kernel (8 NeuronCores, SPMD + AllReduce).

Per core: T/8 triplets.  Device computes Chebyshev/Legendre bases, the
per-pair-type radial einsum (PE matmul with fixed block-diag weights after a
4-way tj/tk one-hot expansion; 4-way ti select on DVE), the outer product
ang = (g_ij*g_ik) (x) P_l, and segment-sums ang into q[20000,8,4] via
gpsimd.dma_scatter_add.  HW scatter-add loses duplicate indices within one
instruction (last-write-wins race), so the host orders each shard's blocks
into occurrence-rank classes (class r = r-th block of an atom on this core):
within a class all atom indices are unique.  Blocks are SEG=16 same-atom
triplet groups formed on the GLOBAL atom-sorted order and dealt round-robin
to cores, which keeps SEG padding global and class sizes balanced.  Classes
are cut into chunks that rotate over KACC DRAM accumulators, so
same-accumulator scatters serialize (WAW dep) while different-accumulator
scatters overlap.  Padding slots scatter to distinct dummy atom rows
(20000..20479) so one uniform program serves all cores.

The q rows are split into NGRP=4 groups of 32 q-partitions; each group's
K-way add + AllReduce + output DMA runs mid-stream as soon as its scatters
complete, so only the last (quarter-sized) group's reduce sits on the tail.

Layout/pipeline notes:
 - The tj/tk one-hot is FUSED into the Chebyshev recurrence: the recurrence
   state B[h,q,k] = onehot(q)*T_k(x_h)*u_h runs in layout [p, h, q, k, J]
   (J innermost => all DVE ops are long-run stride-1 bf16, 2x mode).  The
   PE transpose reads pair-column c via the strided AP [p, feat(64), jj(2)]
   so the (feat,jj) row interleave costs nothing on DVE.
 - W output columns are (jj, h, ti, d) with d innermost; the ti-select adds
   are contiguous 8-wide slices.  q column order is (l,d); the host
   transposes to (d,l) for free.
 - Legendre P is l-major [p, l, J]; P_0==1 is never materialized (the l=0
   partials pre-reduce h itself).  Pre-reduce trees pair contiguous halves
   so every level keeps >=32-element runs.
 - Per macro the PE work is 8 batches of (8 transposes -> 1 ACT cin copy ->
   8 matmuls -> 1 ACT cnd copy); DVE emits next macro's feature build before
   this macro's post-processing so it never waits on PE/ACT.
"""
import sys

sys.path.insert(0, "/opt/trn_rl_repo")
import numpy as np

from concourse import bass, bacc, mybir, tile
from concourse.bass_utils import run_bass_kernel_spmd

N_TYPES, N_DESC, K_MAX, L_MAX = 4, 8, 8, 4
R_C = 5.0
N_ATOMS = 20000
N_CORES = 8
DL = N_DESC * L_MAX          # 32
QPAD = 20480                 # 128 * 160
STRIDE = 64                  # q row stride in f32 (256B; scatter needs 256B mult)
J = 128                      # field columns per macro-tile
MACRO = 128 * J              # 16384 triplets per macro
KACC = 3                     # rotating DRAM accumulators
SEG = 16                     # triplets pre-reduced per scattered block
MB = MACRO // SEG            # blocks per macro (1024)
CHUNK = 1024                 # max idxs (blocks) per scatter instruction
NGRP = 4                     # staged reduce groups (32 q-partitions each)
GP = 128 // NGRP             # q partitions per group
F32, BF16, I16 = mybir.dt.float32, mybir.dt.bfloat16, mybir.dt.int16
PI = float(np.pi)


def _host_prep(n_atoms, triplet_index, r_ij, r_ik, cos_theta,
               type_i, type_j, type_k, c_table):
    """Global atom-sort -> SEG-blocks -> deal blocks round-robin to cores ->
    per-core occurrence-rank classes (per reduce group) -> uniform layout.

    Block b of a core lives at partition b%128, columns SEG*(b//128)..+SEG-1
    (block-major columns).  Pad slots use r=r_c so fc=0 => ang=0."""
    import ml_dtypes
    T = triplet_index.shape[0]
    atom_all = np.asarray(triplet_index[:, 0], dtype=np.int64)

    # ---- global blocks ----
    order = np.argsort(atom_all, kind="stable")
    sa = atom_all[order]
    first = np.r_[True, sa[1:] != sa[:-1]]
    idxf = np.where(first)[0]
    counts = np.diff(np.r_[idxf, T])
    uatoms = sa[idxf]
    nblk_per_atom = -(-counts // SEG)
    nblk_tot = int(nblk_per_atom.sum())

    blk_atom = np.repeat(uatoms, nblk_per_atom)
    starts = np.r_[0, np.cumsum(counts)[:-1]]
    blk_rank_g = (np.arange(nblk_tot)
                  - np.repeat(np.r_[0, np.cumsum(nblk_per_atom)[:-1]],
                              nblk_per_atom))
    blk_start = np.repeat(starts, nblk_per_atom) + blk_rank_g * SEG
    blk_cnt = np.minimum(
        np.repeat(counts, nblk_per_atom) - blk_rank_g * SEG, SEG)

    # ---- deal blocks to cores (rotating offset per atom) ----
    core_of = (blk_rank_g + np.repeat(uatoms, nblk_per_atom)) % N_CORES
    rank_c = blk_rank_g // N_CORES

    GA = GP * (QPAD // 128)              # atoms per reduce group (5120)
    grp = np.minimum(blk_atom // GA, NGRP - 1)

    # ---- class sizes (uniform across cores) ----
    maxr = int(rank_c.max()) + 1
    cls_cnt = np.zeros((N_CORES, NGRP, maxr), dtype=np.int64)
    np.add.at(cls_cnt, (core_of, grp, rank_c), 1)
    cls_list = []                        # [padded_blk_count, g, r]; fillers g=None
    o = 0
    g_end = []
    for g in range(NGRP):
        for r in range(maxr):
            mx = int(cls_cnt[:, g, r].max())
            if mx == 0:
                continue
            p = -(-mx // 128) * 128
            cls_list.append([p, g, r])
            o += p
        g_end.append(o)
    fill = (-o) % MB
    if fill:
        cls_list.append([fill, None, None])
        o += fill
    TBLK = o
    nmacro = TBLK // MB
    TPAD = TBLK * SEG
    # reduce group g is complete after macro mg[g]-1 (all its chunks before)
    mg = [min(-(-e // MB), nmacro) for e in g_end]

    # chunk table in blocks: (start_blk, len_blk); chunks never cross class
    # or macro boundaries; filler ranges are never scattered
    chunks = []
    o = 0
    for p, g, r in cls_list:
        if g is not None:
            sblk = 0
            while sblk < p:
                cl = min(CHUNK, p - sblk)
                mstart = (o + sblk) // MB
                if (o + sblk + cl - 1) // MB != mstart:
                    cl = (mstart + 1) * MB - (o + sblk)
                chunks.append((o + sblk, cl))
                sblk += cl
        o += p

    # ---- destination block id per global block ----
    cls_off = {}
    o = 0
    for p, g, r in cls_list:
        if g is not None:
            cls_off[(g, r)] = o
        o += p
    fields_src = dict(r_ij=np.asarray(r_ij, np.float32),
                      r_ik=np.asarray(r_ik, np.float32),
                      ct=np.asarray(cos_theta, np.float32),
                      ti=np.asarray(type_i, np.float32),
                      tj=np.asarray(type_j, np.float32),
                      tk=np.asarray(type_k, np.float32))
    G = TPAD // 128
    cores = []
    for c in range(N_CORES):
        m = core_of == c
        b_atom = blk_atom[m]
        b_g = grp[m]
        b_r = rank_c[m]
        b_start = blk_start[m]
        b_cnt = blk_cnt[m]
        key = b_g * maxr + b_r
        ordk = np.lexsort((np.arange(len(key)), key))
        pos = np.empty(len(key), dtype=np.int64)
        kk = key[ordk]
        kfirst = np.r_[True, kk[1:] != kk[:-1]]
        kidx = np.where(kfirst)[0]
        within = np.arange(len(key)) - np.repeat(kidx, np.diff(np.r_[kidx, len(key)]))
        pos[ordk] = within
        dst_blk = np.array([cls_off[(g, r)] for g, r in zip(b_g, b_r)],
                           dtype=np.int64) + pos

        dev = {}
        for n in fields_src:
            fillv = R_C if n in ("r_ij", "r_ik") else 0.0
            dt = ml_dtypes.bfloat16 if n in ("ti", "tj", "tk") else np.float32
            dev[n] = np.full((128, G), fillv, dtype=dt)
        bidx = np.empty(TBLK, dtype=np.int16)
        bidx[:] = (20000 + (np.arange(TBLK, dtype=np.int64) % 480)).astype(np.int16)
        bidx[dst_blk] = b_atom.astype(np.int16)

        slot_b = np.repeat(dst_blk, b_cnt)
        slot_s = (np.arange(int(b_cnt.sum()))
                  - np.repeat(np.r_[0, np.cumsum(b_cnt)[:-1]], b_cnt))
        src_idx = order[np.repeat(b_start, b_cnt) + slot_s]
        dst_p = slot_b % 128
        dst_c = SEG * (slot_b // 128) + slot_s
        for n in dev:
            dev[n][dst_p, dst_c] = fields_src[n][src_idx]
        arrays = {n: dev[n] for n in dev}
        arrays["idx"] = np.tile(bidx.reshape(TBLK // 16, 16).T, (8, 1)).copy()
        cores.append(arrays)

    # ---- weight table ----
    # basis fold: reference uses (T_k + 1)*u; we feed T_k*u:
    #   c'[d,0] += sum_k c[d,k]
    ctab = np.asarray(c_table, dtype=np.float64).copy()
    ctab[:, :, :, 0] += ctab.sum(axis=3)
    ctab = ctab.astype(np.float32)
    # rows r = 2*f + jj with f = (h,q,k) = h*32+q*8+k; jj in {0,1} selects
    # the pair member (transpose column c covers J-cols c and c+64)
    # cols o = jj*64 + h*32 + d*4 + ti   (ti innermost)
    W4p = np.zeros((128, 128), dtype=np.float32)
    for h in range(2):
        for q in range(4):
            for k in range(8):
                f = h * 32 + q * 8 + k
                for jj in range(2):
                    for d in range(8):
                        for ti in range(4):
                            W4p[2 * f + jj,
                                jj * 64 + h * 32 + d * 4 + ti] = ctab[ti, q, d, k]
    iotaJ = np.tile(np.arange(4, dtype=np.float32)[:, None], (1, J))
    iotaJ = np.tile(iotaJ.reshape(1, 4 * J), (128, 1))   # [128, 4*J] value=q
    consts = dict(w4=W4p, ident=np.eye(128, dtype=np.float32), iotaj=iotaJ)
    return cores, consts, chunks, nmacro, TPAD, tuple(mg)


def _build(chunks, nmacro, TPAD, mg):
    G = TPAD // 128
    nc = bacc.Bacc(None, target_bir_lowering=False, num_devices=N_CORES,
                   dynamic_dma_scratch_size=32768, num_swdge_queues=1)
    P = {}
    for n in ("r_ij", "r_ik", "ct", "ti", "tj", "tk"):
        fdt = BF16 if n in ("ti", "tj", "tk") else F32
        P[n] = nc.declare_dram_parameter(n, [128, G], fdt, isOutput=False)
    P["idx"] = nc.declare_dram_parameter("idx", [128, TPAD // SEG // 16], I16,
                                         isOutput=False)
    P["w4"] = nc.declare_dram_parameter("w4", [128, 128], F32, isOutput=False)
    P["ident"] = nc.declare_dram_parameter("ident", [128, 128], F32, isOutput=False)
    P["iotaj"] = nc.declare_dram_parameter("iotaj", [128, 4 * J], F32,
                                           isOutput=False)
    out_d = nc.declare_dram_parameter("out", [N_ATOMS, DL], F32, isOutput=True)

    qacc = [nc.dram_tensor(f"qacc{k}", [QPAD, STRIDE], F32) for k in range(KACC)]
    bounce_in = nc.dram_tensor("bounce_in", [128, QPAD * DL // 128], F32)
    bounce_out = nc.dram_tensor("bounce_out", [128, QPAD * DL // 128], F32,
                                addr_space="Shared")

    AF = mybir.ActivationFunctionType
    OP = mybir.AluOpType

    with tile.TileContext(nc) as tc:
        with tc.tile_pool(name="const", bufs=1) as cst:
            w4 = cst.tile([128, 128], BF16)
            ident = cst.tile([128, 128], BF16)
            iotaj = cst.tile([128, 4, J], BF16)
            tmpf = cst.tile([128, 4 * J], F32)
            zero = cst.tile([128, 512], F32)
            halfpi = cst.tile([128, 1], F32)
            nc.vector.memset(halfpi[:], PI / 2)
            negone = cst.tile([128, 1], F32)
            nc.vector.memset(negone[:], -1.0)
            nc.sync.dma_start(out=tmpf[:, :128], in_=P["w4"][:])
            nc.vector.tensor_copy(out=w4[:], in_=tmpf[:, :128])
            nc.sync.dma_start(out=tmpf[:, :128], in_=P["ident"][:])
            nc.vector.tensor_copy(out=ident[:], in_=tmpf[:, :128])
            nc.sync.dma_start(out=tmpf[:], in_=P["iotaj"][:])
            nc.vector.tensor_copy(
                out=iotaj[:].rearrange("p q j -> p (q j)"), in_=tmpf[:])
            nc.vector.memset(zero[:], 0.0)
            qf0 = qacc[0].ap().rearrange("(p r) s -> p (r s)", p=128)
            w0 = QPAD * STRIDE // 128
            for i in range(0, w0, 512):
                nc.sync.dma_start(out=qf0[:, i:i + 512],
                                  in_=zero[:, :min(512, w0 - i)])

            with (
                tc.tile_pool(name="fields", bufs=4) as fpool,
                tc.tile_pool(name="idxp", bufs=4) as ipool,
                tc.tile_pool(name="work", bufs=2) as wpool,
                tc.tile_pool(name="f1p", bufs=2) as f1pool,
                tc.tile_pool(name="cinp", bufs=3) as cpool,
                tc.tile_pool(name="cndp", bufs=2) as cndpool,
                tc.tile_pool(name="postp", bufs=1) as ppool,
                tc.tile_pool(name="redp", bufs=2) as redp,
                tc.tile_pool(name="scat", bufs=3) as spool,
                tc.tile_pool(name="ps1", bufs=2, space="PSUM") as ppool1,
                tc.tile_pool(name="ps2", bufs=2, space="PSUM") as ppool2,
            ):
                by_macro = [[] for _ in range(nmacro)]
                for ci, (s, pl) in enumerate(chunks):
                    by_macro[s // MB].append((ci, s % MB, pl))
                NIC = MB // 16   # idx cols per macro

                NR = QPAD // 128
                qv32 = [q.ap().rearrange("(p r) s -> p r s", p=128)[:, :, :DL]
                        for q in qacc]

                def reduce_group(g):
                    """K-way add + AllReduce + output DMA for q partitions
                    [g*GP, (g+1)*GP)."""
                    p0, p1 = g * GP, (g + 1) * GP
                    acc = redp.tile([128, NR, DL], F32, name="acc")
                    nc.sync.dma_start(out=acc[p0:p1], in_=qv32[0][p0:p1])
                    for k in range(1, KACC):
                        nc.gpsimd.dma_start(out=acc[p0:p1], in_=qv32[k][p0:p1],
                                            accum_op=OP.add)
                    nc.sync.dma_start(out=bounce_in.ap()[p0:p1, :],
                                      in_=acc[p0:p1]
                                      .rearrange("p r s -> p (r s)"))
                    nc.gpsimd.collective_compute(
                        "AllReduce", OP.add,
                        replica_groups=[list(range(N_CORES))],
                        ins=[bounce_in.ap()[p0:p1, :].opt()],
                        outs=[bounce_out.ap()[p0:p1, :].opt()])
                    a0 = p0 * NR
                    a1 = min(p1 * NR, N_ATOMS)
                    if a1 > a0:
                        nc.sync.dma_start(
                            out=out_d.ap().rearrange("a c -> (a c)")
                            [a0 * DL:a1 * DL],
                            in_=bounce_out.ap().rearrange("p f -> (p f)")
                            [a0 * DL:a1 * DL])

                def dma_loads(m):
                    fld = {}
                    for n in ("r_ij", "r_ik", "ct", "ti", "tj", "tk"):
                        fdt = BF16 if n in ("ti", "tj", "tk") else F32
                        t = fpool.tile([128, J], fdt, name=f"fld_{n}")
                        nc.sync.dma_start(out=t[:], in_=P[n][:, m * J:(m + 1) * J])
                        fld[n] = t
                    idxs = ipool.tile([128, NIC], I16, name="idxs")
                    nc.sync.dma_start(out=idxs[:],
                                      in_=P["idx"][:, m * NIC:(m + 1) * NIC])
                    return fld, idxs

                def act_pre(fld):
                    """u = 0.5*sin^2(pi/2 - pi*r/(2rc)); s = (r/rc - 1)^2."""
                    u_both = wpool.tile([128, 2, J], BF16, name="u_both")
                    s_both = wpool.tile([128, 2, J], F32, name="s_both")
                    for half, rn in enumerate(("r_ij", "r_ik")):
                        r = fld[rn]
                        utmp = wpool.tile([128, J], F32, name=f"utmp{half}")
                        nc.scalar.activation(utmp[:], r[:], AF.Sin,
                                             bias=halfpi[:], scale=-PI / (2 * R_C))
                        nc.scalar.activation(u_both[:, half, :], utmp[:], AF.Square,
                                             scale=float(np.sqrt(0.5)))
                        nc.scalar.activation(s_both[:, half, :], r[:], AF.Square,
                                             bias=negone[:], scale=1.0 / R_C)
                    ct2 = wpool.tile([128, J], F32, name="ct2")
                    nc.scalar.activation(ct2[:], fld["ct"][:], AF.Square)
                    return u_both, s_both, ct2

                def build_features(fld, u_both, s_both):
                    """B[h,q,k] = oh(q)*T_k(x_h)*u_h, layout [p, h, q, k, J]."""
                    B = f1pool.tile([128, 2, 4, 8, J], BF16, name="B")
                    x_both = wpool.tile([128, 2, J], BF16, name="x_both")
                    x2_both = wpool.tile([128, 2, J], BF16, name="x2_both")
                    nc.vector.tensor_scalar(out=x_both[:], in0=s_both[:],
                                            scalar1=2.0, scalar2=-1.0,
                                            op0=OP.mult, op1=OP.add)
                    nc.vector.tensor_scalar(out=x2_both[:], in0=s_both[:],
                                            scalar1=4.0, scalar2=-2.0,
                                            op0=OP.mult, op1=OP.add)
                    t2 = wpool.tile([128, 2, J], BF16, name="t2")
                    nc.vector.tensor_copy(out=t2[:, 0, :], in_=fld["tj"][:])
                    nc.vector.tensor_copy(out=t2[:, 1, :], in_=fld["tk"][:])
                    oh = wpool.tile([128, 2, 4, J], BF16, name="oh")
                    nc.vector.tensor_tensor(
                        out=oh[:],
                        in0=t2[:].unsqueeze(2).broadcast_to([128, 2, 4, J]),
                        in1=iotaj[:].unsqueeze(1).broadcast_to([128, 2, 4, J]),
                        op=OP.is_equal)
                    ub = u_both[:].unsqueeze(2).broadcast_to([128, 2, 4, J])
                    xb = x_both[:].unsqueeze(2).broadcast_to([128, 2, 4, J])
                    x2b = x2_both[:].unsqueeze(2).broadcast_to([128, 2, 4, J])
                    nc.vector.tensor_tensor(out=B[:, :, :, 0, :], in0=oh[:],
                                            in1=ub, op=OP.mult)
                    nc.vector.tensor_tensor(out=B[:, :, :, 1, :],
                                            in0=B[:, :, :, 0, :],
                                            in1=xb, op=OP.mult)
                    for k in range(2, 8):
                        nc.vector.tensor_tensor(out=B[:, :, :, k, :],
                                                in0=B[:, :, :, k - 1, :],
                                                in1=x2b, op=OP.mult)
                        nc.vector.tensor_tensor(out=B[:, :, :, k, :],
                                                in0=B[:, :, :, k, :],
                                                in1=B[:, :, :, k - 2, :],
                                                op=OP.subtract)
                    return B

                def pe_stage(B):
                    """8 batches of (8 transposes -> cin copy -> 8 matmuls ->
                    cnd copy).  Transpose input for pair-col c is the strided
                    AP [p, feat(64), jj(2)] -> loaded rows r = 2f+jj."""
                    # J = (jj, c): pair-column c holds J-cols c and c+64, so
                    # the 128 pair values sit at a single stride of 64:
                    # address = c + 64*(2f + jj)
                    Bj = B[:].rearrange("p h q k (jj c) -> p (h q k jj) c",
                                        jj=2)
                    cndb = cndpool.tile([128, 64, 128], BF16, name="cndb")

                    def mm_batch(bb, cin):
                        ps2 = ppool2.tile([128, 8, 128], F32, space="PSUM",
                                          name="ps2")
                        for jcol in range(8):
                            nc.tensor.matmul(out=ps2[:, jcol, :],
                                             lhsT=cin[:, jcol, :],
                                             rhs=w4[:], start=True, stop=True)
                        nc.scalar.activation(
                            cndb[:, 8 * bb:8 * bb + 8, :]
                            .rearrange("p c f -> p (c f)"),
                            ps2[:].rearrange("p c f -> p (c f)"), AF.Identity)

                    prev = None
                    for b in range(8):
                        ps1 = ppool1.tile([128, 8, 128], BF16, space="PSUM",
                                          name="ps1")
                        for jcol in range(8):
                            c = 8 * b + jcol
                            nc.tensor.transpose(
                                out=ps1[:, jcol, :],
                                in_=Bj[:, :, c],
                                identity=ident[:])
                        cin = cpool.tile([128, 8, 128], BF16, name="cin")
                        nc.scalar.activation(
                            cin[:].rearrange("p c f -> p (c f)"),
                            ps1[:].rearrange("p c f -> p (c f)"), AF.Identity)
                        if prev is not None:
                            mm_batch(prev[0], prev[1])
                        prev = (b, cin)
                    mm_batch(prev[0], prev[1])
                    return cndb

                def post_stage(cndb, fld, ct2):
                    """ti-select + product + P_l outer + SEG pre-reduce."""
                    oh_ti = wpool.tile([128, J, 4], BF16, name="oh_ti")
                    nc.vector.tensor_tensor(
                        out=oh_ti[:],
                        in0=fld["ti"][:].unsqueeze(2).broadcast_to([128, J, 4]),
                        in1=iotaj[:, :, 0].unsqueeze(1)
                            .broadcast_to([128, J, 4]),
                        op=OP.is_equal)
                    # cnd cols o = jj*64 + h*32 + d*4 + ti; row c of cndb is
                    # pair-col c = (J-cols c, c+64)  ->  field j = jj*64 + c
                    cv = cndb[:].rearrange("p c (jj h f) -> p jj c h f",
                                           jj=2, h=2)
                    sel0 = ppool.tile([128, 2, 64, 32], BF16, name="sel0")
                    ohv = oh_ti[:].rearrange("p (jj c) t -> p jj c t", jj=2)
                    for jj in range(2):
                        nc.vector.tensor_tensor(
                            out=sel0[:, jj].rearrange("p c (d t) -> p c d t", t=4),
                            in0=cv[:, jj, :, 0, :]
                                .rearrange("p c (d t) -> p c d t", t=4),
                            in1=ohv[:, jj].unsqueeze(2)
                                .broadcast_to([128, 64, 8, 4]),
                            op=OP.mult)
                    prod = ppool.tile([128, J, 32], BF16, name="prod")
                    prodv = prod[:].rearrange("p (jj c) f -> p jj c f", jj=2)
                    for jj in range(2):
                        nc.vector.tensor_tensor(
                            out=prodv[:, jj],
                            in0=sel0[:, jj],
                            in1=cv[:, jj, :, 1, :], op=OP.mult)
                    # sum over ti (innermost 4)
                    pv = prod[:].rearrange("p j (d a b) -> p j d a b", a=2, b=2)
                    a0 = ppool.tile([128, J, 8, 2], BF16, name="a0")
                    nc.vector.tensor_tensor(out=a0[:], in0=pv[:, :, :, 0, :],
                                            in1=pv[:, :, :, 1, :], op=OP.add)
                    h8 = ppool.tile([128, J, 8], BF16, name="h8")
                    nc.vector.tensor_tensor(out=h8[:], in0=a0[:, :, :, 0],
                                            in1=a0[:, :, :, 1], op=OP.add)
                    # Legendre P l-major [p, 3, J]
                    Pt = wpool.tile([128, 3, J], BF16, name="Pt")
                    nc.vector.tensor_copy(out=Pt[:, 0, :], in_=fld["ct"][:])
                    nc.vector.tensor_scalar(out=Pt[:, 1, :], in0=ct2[:],
                                            scalar1=1.5, scalar2=-0.5,
                                            op0=OP.mult, op1=OP.add)
                    p3t = wpool.tile([128, J], BF16, name="p3t")
                    nc.vector.tensor_scalar(out=p3t[:], in0=ct2[:],
                                            scalar1=2.5, scalar2=-1.5,
                                            op0=OP.mult, op1=OP.add)
                    nc.vector.tensor_tensor(out=Pt[:, 2, :], in0=p3t[:],
                                            in1=Pt[:, 0, :], op=OP.mult)
                    # ang for l=1..3 [p, J, 3, 8]
                    ang = ppool.tile([128, J, 3, 8], BF16, name="ang")
                    nc.vector.tensor_tensor(
                        out=ang[:],
                        in0=Pt[:].rearrange("p l j -> p j l").unsqueeze(3)
                            .broadcast_to([128, J, 3, 8]),
                        in1=h8[:].unsqueeze(2).broadcast_to([128, J, 3, 8]),
                        op=OP.mult)
                    # SEG=16 pre-reduce by contiguous halves;
                    # partials [p, MB/128, (l,d)=32] f32, l=0 from h8.
                    partials = spool.tile([128, MB // 128, 32], F32,
                                          name="partials")
                    NB = MB // 128      # blocks per partition per macro (8)
                    hv = h8[:].rearrange("p (g s) d -> p g s d", g=NB)
                    t1 = ppool.tile([128, NB, 8, 8], BF16, name="t1")
                    nc.vector.tensor_tensor(out=t1[:], in0=hv[:, :, 0:8, :],
                                            in1=hv[:, :, 8:16, :], op=OP.add)
                    t2s = ppool.tile([128, NB, 4, 8], BF16, name="t2s")
                    nc.vector.tensor_tensor(out=t2s[:], in0=t1[:, :, 0:4, :],
                                            in1=t1[:, :, 4:8, :], op=OP.add)
                    t3 = ppool.tile([128, NB, 2, 8], BF16, name="t3")
                    nc.vector.tensor_tensor(out=t3[:], in0=t2s[:, :, 0:2, :],
                                            in1=t2s[:, :, 2:4, :], op=OP.add)
                    nc.vector.tensor_tensor(out=partials[:, :, 0:8],
                                            in0=t3[:, :, 0, :],
                                            in1=t3[:, :, 1, :], op=OP.add)
                    av = ang[:].rearrange("p (g s) l d -> p g s (l d)", g=NB)
                    u1 = ppool.tile([128, NB, 8, 24], BF16, name="u1")
                    nc.vector.tensor_tensor(out=u1[:], in0=av[:, :, 0:8, :],
                                            in1=av[:, :, 8:16, :], op=OP.add)
                    u2 = ppool.tile([128, NB, 4, 24], BF16, name="u2")
                    nc.vector.tensor_tensor(out=u2[:], in0=u1[:, :, 0:4, :],
                                            in1=u1[:, :, 4:8, :], op=OP.add)
                    u3 = ppool.tile([128, NB, 2, 24], BF16, name="u3")
                    nc.vector.tensor_tensor(out=u3[:], in0=u2[:, :, 0:2, :],
                                            in1=u2[:, :, 2:4, :], op=OP.add)
                    nc.vector.tensor_tensor(out=partials[:, :, 8:32],
                                            in0=u3[:, :, 0, :],
                                            in1=u3[:, :, 1, :], op=OP.add)
                    return partials

                ld = {0: dma_loads(0)}
                if nmacro > 1:
                    ld[1] = dma_loads(1)
                ap = {0: act_pre(ld[0][0])}
                F1cur = build_features(ld[0][0], ap[0][0], ap[0][1])
                for k in range(1, KACC):
                    qf = qacc[k].ap().rearrange("(p r) s -> p (r s)", p=128)
                    w = QPAD * STRIDE // 128
                    for i in range(0, w, 512):
                        nc.sync.dma_start(out=qf[:, i:i + 512],
                                          in_=zero[:, :min(512, w - i)])

                # reduce group g: emitted after macro mg[g] (one extra macro
                # of slack so its scatter drains are done before the load)
                red_at = {}
                for g in range(NGRP):
                    mm = min(mg[g] + 1, nmacro - 1) if g < NGRP - 1 else nmacro
                    red_at.setdefault(mm, []).append(g)

                for m in range(nmacro):
                    fld, idxs = ld[m]
                    u_both, s_both, ct2 = ap[m]
                    F1use = F1cur
                    if m + 2 < nmacro:
                        ld[m + 2] = dma_loads(m + 2)
                    if m + 1 < nmacro:
                        ap[m + 1] = act_pre(ld[m + 1][0])
                        F1cur = build_features(ld[m + 1][0], ap[m + 1][0],
                                               ap[m + 1][1])
                    cndb = pe_stage(F1use)
                    partials = post_stage(cndb, fld, ct2)
                    ld.pop(m, None)
                    ap.pop(m, None)

                    for (ci, off, pl) in by_macro[m]:
                        nc.gpsimd.dma_scatter_add(
                            qacc[ci % KACC].ap()[:, :DL],
                            partials[:, off // 128:(off + pl) // 128, :],
                            idxs[:, off // 16:(off + pl) // 16],
                            pl, pl, DL, elem_step=STRIDE,
                            queue_num=0)

                    for g in red_at.get(m, []):
                        reduce_group(g)

                for g in red_at.get(nmacro, []):
                    reduce_group(g)
    nc.compile()
    return nc


def _install_ntff_hook():
    """Provide antenv.axon_hooks (missing in this image) via sys.modules so
    run_bass_kernel_spmd(trace=True) can capture NTFF profiles."""
    import types, ctypes, contextlib
    try:
        from antenv.axon_hooks import get_axon_ntff_profile_hook  # noqa: F401
        return
    except ImportError:
        pass
    so_path = "/opt/axon/libaxon_pjrt.so"
    try:
        lib = ctypes.CDLL(so_path)
    except OSError:
        return
    if not hasattr(lib, "axon_start_nrt_profile"):
        return
    lib.axon_start_nrt_profile.argtypes = [ctypes.POINTER(ctypes.c_int64),
                                           ctypes.c_size_t]
    lib.axon_start_nrt_profile.restype = ctypes.c_int64
    lib.axon_stop_nrt_profile.argtypes = [ctypes.c_char_p]
    lib.axon_stop_nrt_profile.restype = ctypes.c_int64

    @contextlib.contextmanager
    def _hook(output_dir, device_ids):
        import jax
        jax.devices()
        if device_ids:
            ids = (ctypes.c_int64 * len(device_ids))(*device_ids)
            rc = lib.axon_start_nrt_profile(ids, len(device_ids))
        else:
            rc = lib.axon_start_nrt_profile(None, 0)
        if rc != 0:
            raise RuntimeError(f"axon_start_nrt_profile rc={rc}")
        try:
            yield
        finally:
            n = lib.axon_stop_nrt_profile(str(output_dir).encode())
            if n <= 0:
                print(f"ntff capture wrote {n} files", flush=True)

    mod = types.ModuleType("antenv.axon_hooks")
    mod.get_axon_ntff_profile_hook = lambda: _hook
    mod.set_axon_ntff_profile_hook = lambda h: None
    import antenv
    sys.modules["antenv.axon_hooks"] = mod
    antenv.axon_hooks = mod


_CACHE = {}


def kernel(n_atoms, triplet_index, r_ij, r_ik, cos_theta,
           type_i, type_j, type_k, c_table, _sim=False, _trace=False):
    cores, consts, chunks, nmacro, TPAD, mg = _host_prep(
        n_atoms, triplet_index, r_ij, r_ik, cos_theta,
        type_i, type_j, type_k, c_table)
    key = (nmacro, TPAD, mg, tuple(chunks))
    if key not in _CACHE:
        _CACHE[key] = _build(chunks, nmacro, TPAD, mg)
    nc = _CACHE[key]
    in_maps = []
    for c in range(N_CORES):
        m = dict(cores[c])
        m.update(consts)
        in_maps.append(m)
    if _sim:
        from concourse import bass_interp
        sim = bass_interp.MultiCoreSim(nc, N_CORES)
        for c in range(N_CORES):
            for k, v in in_maps[c].items():
                sim.cores[c].tensor(k)[:] = v
        sim.simulate()
        out = np.array(sim.cores[0].mem_tensor("out"))
    else:
        if _trace:
            _install_ntff_hook()
        last_err = None
        for _try in range(3):
            try:
                res = run_bass_kernel_spmd(nc, in_maps,
                                           core_ids=list(range(N_CORES)),
                                           trace=_trace)
                out = np.asarray(res.results[0]["out"])
                break
            except Exception as e:  # transient device-unrecoverable after a crash
                last_err = e
        else:
            raise last_err
        kernel.last_exec_ns = res.exec_time_ns
        kernel.last_results = res
    # device q columns are (l, d); reference wants (d, l)
    return (out.reshape(N_ATOMS, L_MAX, N_DESC).transpose(0, 2, 1)
            .astype(np.float32))


# revision 15
# speedup vs baseline: 1.7836x; 1.0361x over previous
"""AngularDescriptor Trainium2 kernel (8 NeuronCores, SPMD + AllReduce).

Per core: T/8 triplets.  Device computes Chebyshev/Legendre bases, the
per-pair-type radial einsum (PE matmul with fixed block-diag weights after a
4-way tj/tk one-hot expansion; 4-way ti select on DVE), the outer product
ang = (g_ij*g_ik) (x) P_l, and segment-sums ang into q[20000,8,4] via
gpsimd.dma_scatter_add.  HW scatter-add loses duplicate indices within one
instruction (last-write-wins race), so the host orders each shard's blocks
into occurrence-rank classes (class r = r-th block of an atom on this core):
within a class all atom indices are unique.  Blocks are SEG=16 same-atom
triplet groups formed on the GLOBAL atom-sorted order and dealt round-robin
to cores, which keeps SEG padding global and class sizes balanced.  Classes
are cut into chunks that rotate over KACC DRAM accumulators, so
same-accumulator scatters serialize (WAW dep) while different-accumulator
scatters overlap.  Padding slots scatter to distinct dummy atom rows
(20000..20479) so one uniform program serves all cores.

The q rows are split into NGRP=4 groups of 32 q-partitions; each group's
K-way add + AllReduce + output DMA runs mid-stream as soon as its scatters
complete, so only the last (quarter-sized) group's reduce sits on the tail.

Layout/pipeline notes:
 - The tj/tk one-hot is FUSED into the Chebyshev recurrence: the recurrence
   state B[h,q,k] = onehot(q)*T_k(x_h)*u_h runs in layout [p, h, q, k, J]
   (J innermost => all DVE ops are long-run stride-1 bf16, 2x mode).  The
   PE transpose reads pair-column c via the strided AP [p, feat(64), jj(2)]
   so the (feat,jj) row interleave costs nothing on DVE.
 - W output columns are (jj, h, ti, d) with d innermost; the ti-select adds
   are contiguous 8-wide slices.  q column order is (l,d); the host
   transposes to (d,l) for free.
 - Legendre P is l-major [p, l, J]; P_0==1 is never materialized (the l=0
   partials pre-reduce h itself).  Pre-reduce trees pair contiguous halves
   so every level keeps >=32-element runs.
 - Per macro the PE work is 8 batches of (8 transposes -> 1 ACT cin copy ->
   8 matmuls -> 1 ACT cnd copy); DVE emits next macro's feature build before
   this macro's post-processing so it never waits on PE/ACT.
"""
import sys

sys.path.insert(0, "/opt/trn_rl_repo")
import numpy as np

from concourse import bass, bacc, mybir, tile
from concourse.bass_utils import run_bass_kernel_spmd

N_TYPES, N_DESC, K_MAX, L_MAX = 4, 8, 8, 4
R_C = 5.0
N_ATOMS = 20000
N_CORES = 8
DL = N_DESC * L_MAX          # 32
QPAD = 20480                 # 128 * 160
STRIDE = 64                  # q row stride in f32 (256B; scatter needs 256B mult)
J = 128                      # field columns per macro-tile
MACRO = 128 * J              # 16384 triplets per macro
KACC = 3                     # rotating DRAM accumulators
SEG = 16                     # triplets pre-reduced per scattered block
MB = MACRO // SEG            # blocks per macro (1024)
CHUNK = 1024                 # max idxs (blocks) per scatter instruction
NGRP = 4                     # staged reduce groups (32 q-partitions each)
GP = 128 // NGRP             # q partitions per group
F32, BF16, I16 = mybir.dt.float32, mybir.dt.bfloat16, mybir.dt.int16
PI = float(np.pi)


def _host_prep(n_atoms, triplet_index, r_ij, r_ik, cos_theta,
               type_i, type_j, type_k, c_table):
    """Global atom-sort -> SEG-blocks -> deal blocks round-robin to cores ->
    per-core occurrence-rank classes (per reduce group) -> uniform layout.

    Block b of a core lives at partition b%128, columns SEG*(b//128)..+SEG-1
    (block-major columns).  Pad slots use r=r_c so fc=0 => ang=0."""
    import ml_dtypes
    T = triplet_index.shape[0]
    atom_all = np.asarray(triplet_index[:, 0], dtype=np.int64)

    # ---- global blocks ----
    order = np.argsort(atom_all, kind="stable")
    sa = atom_all[order]
    first = np.r_[True, sa[1:] != sa[:-1]]
    idxf = np.where(first)[0]
    counts = np.diff(np.r_[idxf, T])
    uatoms = sa[idxf]
    nblk_per_atom = -(-counts // SEG)
    nblk_tot = int(nblk_per_atom.sum())

    blk_atom = np.repeat(uatoms, nblk_per_atom)
    starts = np.r_[0, np.cumsum(counts)[:-1]]
    blk_rank_g = (np.arange(nblk_tot)
                  - np.repeat(np.r_[0, np.cumsum(nblk_per_atom)[:-1]],
                              nblk_per_atom))
    blk_start = np.repeat(starts, nblk_per_atom) + blk_rank_g * SEG
    blk_cnt = np.minimum(
        np.repeat(counts, nblk_per_atom) - blk_rank_g * SEG, SEG)

    # ---- deal blocks to cores (rotating offset per atom) ----
    core_of = (blk_rank_g + np.repeat(uatoms, nblk_per_atom)) % N_CORES
    rank_c = blk_rank_g // N_CORES

    GA = GP * (QPAD // 128)              # atoms per reduce group (5120)
    grp = np.minimum(blk_atom // GA, NGRP - 1)

    # ---- class sizes (uniform across cores) ----
    maxr = int(rank_c.max()) + 1
    cls_cnt = np.zeros((N_CORES, NGRP, maxr), dtype=np.int64)
    np.add.at(cls_cnt, (core_of, grp, rank_c), 1)
    cls_list = []                        # [padded_blk_count, g, r]; fillers g=None
    o = 0
    g_end = []
    for g in range(NGRP):
        for r in range(maxr):
            mx = int(cls_cnt[:, g, r].max())
            if mx == 0:
                continue
            p = -(-mx // 128) * 128
            cls_list.append([p, g, r])
            o += p
        g_end.append(o)
    fill = (-o) % MB
    if fill:
        cls_list.append([fill, None, None])
        o += fill
    TBLK = o
    nmacro = TBLK // MB
    TPAD = TBLK * SEG
    # reduce group g is complete after macro mg[g]-1 (all its chunks before)
    mg = [min(-(-e // MB), nmacro) for e in g_end]

    # chunk table in blocks: (start_blk, len_blk); chunks never cross class
    # or macro boundaries; filler ranges are never scattered
    chunks = []
    o = 0
    for p, g, r in cls_list:
        if g is not None:
            sblk = 0
            while sblk < p:
                cl = min(CHUNK, p - sblk)
                mstart = (o + sblk) // MB
                if (o + sblk + cl - 1) // MB != mstart:
                    cl = (mstart + 1) * MB - (o + sblk)
                chunks.append((o + sblk, cl))
                sblk += cl
        o += p

    # ---- destination block id per global block ----
    cls_off = {}
    o = 0
    for p, g, r in cls_list:
        if g is not None:
            cls_off[(g, r)] = o
        o += p
    fields_src = dict(r_ij=np.asarray(r_ij, np.float32),
                      r_ik=np.asarray(r_ik, np.float32),
                      ct=np.asarray(cos_theta, np.float32),
                      ti=np.asarray(type_i, np.float32),
                      tj=np.asarray(type_j, np.float32),
                      tk=np.asarray(type_k, np.float32))
    G = TPAD // 128
    cores = []
    for c in range(N_CORES):
        m = core_of == c
        b_atom = blk_atom[m]
        b_g = grp[m]
        b_r = rank_c[m]
        b_start = blk_start[m]
        b_cnt = blk_cnt[m]
        key = b_g * maxr + b_r
        ordk = np.lexsort((np.arange(len(key)), key))
        pos = np.empty(len(key), dtype=np.int64)
        kk = key[ordk]
        kfirst = np.r_[True, kk[1:] != kk[:-1]]
        kidx = np.where(kfirst)[0]
        within = np.arange(len(key)) - np.repeat(kidx, np.diff(np.r_[kidx, len(key)]))
        pos[ordk] = within
        dst_blk = np.array([cls_off[(g, r)] for g, r in zip(b_g, b_r)],
                           dtype=np.int64) + pos

        dev = {}
        for n in fields_src:
            fillv = R_C if n in ("r_ij", "r_ik") else 0.0
            dt = ml_dtypes.bfloat16 if n in ("ti", "tj", "tk") else np.float32
            dev[n] = np.full((128, G), fillv, dtype=dt)
        # q row for atom a is (a%128)*160 + a//128: each reduce group's
        # atoms [5120g, 5120(g+1)) form row range [40g, 40g+40) on ALL 128
        # partitions, so the staged collectives run full-width.
        bidx = np.empty(TBLK, dtype=np.int16)
        dum = 20000 + (np.arange(TBLK, dtype=np.int64) % 480)
        bidx[:] = ((dum % 128) * 160 + dum // 128).astype(np.int16)
        bidx[dst_blk] = ((b_atom % 128) * 160 + b_atom // 128).astype(np.int16)

        slot_b = np.repeat(dst_blk, b_cnt)
        slot_s = (np.arange(int(b_cnt.sum()))
                  - np.repeat(np.r_[0, np.cumsum(b_cnt)[:-1]], b_cnt))
        src_idx = order[np.repeat(b_start, b_cnt) + slot_s]
        dst_p = slot_b % 128
        dst_c = SEG * (slot_b // 128) + slot_s
        for n in dev:
            dev[n][dst_p, dst_c] = fields_src[n][src_idx]
        arrays = {n: dev[n] for n in dev}
        arrays["idx"] = np.tile(bidx.reshape(TBLK // 16, 16).T, (8, 1)).copy()
        cores.append(arrays)

    # ---- weight table ----
    # basis fold: reference uses (T_k + 1)*u; we feed T_k*u:
    #   c'[d,0] += sum_k c[d,k]
    ctab = np.asarray(c_table, dtype=np.float64).copy()
    ctab[:, :, :, 0] += ctab.sum(axis=3)
    ctab = ctab.astype(np.float32)
    # rows r = 2*f + jj with f = (h,q,k) = h*32+q*8+k; jj in {0,1} selects
    # the pair member (transpose column c covers J-cols c and c+64)
    # cols o = jj*64 + h*32 + d*4 + ti   (ti innermost)
    W4p = np.zeros((128, 128), dtype=np.float32)
    for h in range(2):
        for q in range(4):
            for k in range(8):
                f = h * 32 + q * 8 + k
                for jj in range(2):
                    for d in range(8):
                        for ti in range(4):
                            W4p[2 * f + jj,
                                jj * 64 + h * 32 + d * 4 + ti] = ctab[ti, q, d, k]
    iotaJ = np.tile(np.arange(4, dtype=np.float32)[:, None], (1, J))
    iotaJ = np.tile(iotaJ.reshape(1, 4 * J), (128, 1))   # [128, 4*J] value=q
    consts = dict(w4=W4p, ident=np.eye(128, dtype=np.float32), iotaj=iotaJ)
    return cores, consts, chunks, nmacro, TPAD, tuple(mg)


def _build(chunks, nmacro, TPAD, mg):
    G = TPAD // 128
    nc = bacc.Bacc(None, target_bir_lowering=False, num_devices=N_CORES,
                   dynamic_dma_scratch_size=32768, num_swdge_queues=1)
    P = {}
    for n in ("r_ij", "r_ik", "ct", "ti", "tj", "tk"):
        fdt = BF16 if n in ("ti", "tj", "tk") else F32
        P[n] = nc.declare_dram_parameter(n, [128, G], fdt, isOutput=False)
    P["idx"] = nc.declare_dram_parameter("idx", [128, TPAD // SEG // 16], I16,
                                         isOutput=False)
    P["w4"] = nc.declare_dram_parameter("w4", [128, 128], F32, isOutput=False)
    P["ident"] = nc.declare_dram_parameter("ident", [128, 128], F32, isOutput=False)
    P["iotaj"] = nc.declare_dram_parameter("iotaj", [128, 4 * J], F32,
                                           isOutput=False)
    out_d = nc.declare_dram_parameter("out", [128, QPAD * DL // 128], F32,
                                      isOutput=True)

    qacc = [nc.dram_tensor(f"qacc{k}", [QPAD, STRIDE], F32) for k in range(KACC)]
    GCOL = QPAD * DL // 128 // NGRP
    bnc_in = [nc.dram_tensor(f"bounce_in{g}", [128, GCOL], F32)
              for g in range(NGRP)]
    bnc_out = [nc.dram_tensor(f"bounce_out{g}", [128, GCOL], F32,
                              addr_space="Shared") for g in range(NGRP)]

    AF = mybir.ActivationFunctionType
    OP = mybir.AluOpType

    with tile.TileContext(nc) as tc:
        with tc.tile_pool(name="const", bufs=1) as cst:
            w4 = cst.tile([128, 128], BF16)
            ident = cst.tile([128, 128], BF16)
            iotaj = cst.tile([128, 4, J], BF16)
            tmpf = cst.tile([128, 4 * J], F32)
            zero = cst.tile([128, 512], F32)
            halfpi = cst.tile([128, 1], F32)
            nc.vector.memset(halfpi[:], PI / 2)
            negone = cst.tile([128, 1], F32)
            nc.vector.memset(negone[:], -1.0)
            neghalf = cst.tile([128, 1], F32)
            nc.vector.memset(neghalf[:], -0.5)
            neg32 = cst.tile([128, 1], F32)
            nc.vector.memset(neg32[:], -1.5)
            nc.sync.dma_start(out=tmpf[:, :128], in_=P["w4"][:])
            nc.vector.tensor_copy(out=w4[:], in_=tmpf[:, :128])
            nc.sync.dma_start(out=tmpf[:, :128], in_=P["ident"][:])
            nc.vector.tensor_copy(out=ident[:], in_=tmpf[:, :128])
            nc.sync.dma_start(out=tmpf[:], in_=P["iotaj"][:])
            nc.vector.tensor_copy(
                out=iotaj[:].rearrange("p q j -> p (q j)"), in_=tmpf[:])
            nc.vector.memset(zero[:], 0.0)
            qf0 = qacc[0].ap().rearrange("(p r) s -> p (r s)", p=128)
            w0 = QPAD * STRIDE // 128
            for i in range(0, w0, 512):
                nc.sync.dma_start(out=qf0[:, i:i + 512],
                                  in_=zero[:, :min(512, w0 - i)])

            with (
                tc.tile_pool(name="fields", bufs=4) as fpool,
                tc.tile_pool(name="idxp", bufs=4) as ipool,
                tc.tile_pool(name="work", bufs=2) as wpool,
                tc.tile_pool(name="f1p", bufs=2) as f1pool,
                tc.tile_pool(name="cinp", bufs=3) as cpool,
                tc.tile_pool(name="cndp", bufs=2) as cndpool,
                tc.tile_pool(name="postp", bufs=1) as ppool,
                tc.tile_pool(name="redp", bufs=2) as redp,
                tc.tile_pool(name="scat", bufs=3) as spool,
                tc.tile_pool(name="ps1", bufs=2, space="PSUM") as ppool1,
                tc.tile_pool(name="ps2", bufs=2, space="PSUM") as ppool2,
            ):
                by_macro = [[] for _ in range(nmacro)]
                for ci, (s, pl) in enumerate(chunks):
                    by_macro[s // MB].append((ci, s % MB, pl))
                NIC = MB // 16   # idx cols per macro

                NR = QPAD // 128
                qv32 = [q.ap().rearrange("(p r) s -> p r s", p=128)[:, :, :DL]
                        for q in qacc]

                def reduce_group(g):
                    """K-way add + AllReduce + output DMA for q rows
                    [g*NR/NGRP, (g+1)*NR/NGRP) on all 128 partitions."""
                    r0, r1 = g * (NR // NGRP), (g + 1) * (NR // NGRP)
                    acc = redp.tile([128, NR // NGRP, DL], F32, name="acc")
                    nc.sync.dma_start(out=acc[:], in_=qv32[0][:, r0:r1])
                    for k in range(1, KACC):
                        nc.gpsimd.dma_start(out=acc[:], in_=qv32[k][:, r0:r1],
                                            accum_op=OP.add)
                    c0, c1 = r0 * DL, r1 * DL
                    nc.sync.dma_start(out=bnc_in[g].ap()[:],
                                      in_=acc[:]
                                      .rearrange("p r s -> p (r s)"))
                    nc.gpsimd.collective_compute(
                        "AllReduce", OP.add,
                        replica_groups=[list(range(N_CORES))],
                        ins=[bnc_in[g].ap()[:].opt()],
                        outs=[bnc_out[g].ap()[:].opt()])
                    nc.sync.dma_start(out=out_d.ap()[:, c0:c1],
                                      in_=bnc_out[g].ap()[:])

                def dma_loads(m):
                    fld = {}
                    for n in ("r_ij", "r_ik", "ct", "ti", "tj", "tk"):
                        fdt = BF16 if n in ("ti", "tj", "tk") else F32
                        t = fpool.tile([128, J], fdt, name=f"fld_{n}")
                        nc.sync.dma_start(out=t[:], in_=P[n][:, m * J:(m + 1) * J])
                        fld[n] = t
                    idxs = ipool.tile([128, NIC], I16, name="idxs")
                    nc.sync.dma_start(out=idxs[:],
                                      in_=P["idx"][:, m * NIC:(m + 1) * NIC])
                    return fld, idxs

                def act_pre(fld):
                    """u = 0.5*sin^2(pi/2 - pi*r/(2rc)); s = (r/rc - 1)^2."""
                    u_both = wpool.tile([128, 2, J], BF16, name="u_both")
                    s_both = wpool.tile([128, 2, J], F32, name="s_both")
                    for half, rn in enumerate(("r_ij", "r_ik")):
                        r = fld[rn]
                        utmp = wpool.tile([128, J], F32, name=f"utmp{half}")
                        nc.scalar.activation(utmp[:], r[:], AF.Sin,
                                             bias=halfpi[:], scale=-PI / (2 * R_C))
                        nc.scalar.activation(u_both[:, half, :], utmp[:], AF.Square,
                                             scale=float(np.sqrt(0.5)))
                        nc.scalar.activation(s_both[:, half, :], r[:], AF.Square,
                                             bias=negone[:], scale=1.0 / R_C)
                    # Legendre P l-major [p, 3, J]: P1=ct, P2=1.5ct2-.5,
                    # p3t=2.5ct2-1.5 (P3 = p3t*ct on DVE later)
                    ct2 = wpool.tile([128, J], F32, name="ct2")
                    nc.scalar.activation(ct2[:], fld["ct"][:], AF.Square)
                    Pt = wpool.tile([128, 3, J], BF16, name="Pt")
                    nc.scalar.activation(Pt[:, 0, :], fld["ct"][:], AF.Identity)
                    nc.scalar.activation(Pt[:, 1, :], ct2[:], AF.Identity,
                                         bias=neghalf[:], scale=1.5)
                    p3t = wpool.tile([128, J], BF16, name="p3t")
                    nc.scalar.activation(p3t[:], ct2[:], AF.Identity,
                                         bias=neg32[:], scale=2.5)
                    return u_both, s_both, (Pt, p3t)

                def build_features(fld, u_both, s_both):
                    """B[h,q,k] = oh(q)*T_k(x_h)*u_h, layout [p, h, q, k, J]."""
                    B = f1pool.tile([128, 2, 4, 8, J], BF16, name="B")
                    x_both = wpool.tile([128, 2, J], BF16, name="x_both")
                    x2_both = wpool.tile([128, 2, J], BF16, name="x2_both")
                    nc.vector.tensor_scalar(out=x_both[:], in0=s_both[:],
                                            scalar1=2.0, scalar2=-1.0,
                                            op0=OP.mult, op1=OP.add)
                    nc.vector.tensor_scalar(out=x2_both[:], in0=s_both[:],
                                            scalar1=4.0, scalar2=-2.0,
                                            op0=OP.mult, op1=OP.add)
                    t2 = wpool.tile([128, 2, J], BF16, name="t2")
                    nc.vector.tensor_copy(out=t2[:, 0, :], in_=fld["tj"][:])
                    nc.vector.tensor_copy(out=t2[:, 1, :], in_=fld["tk"][:])
                    oh = wpool.tile([128, 2, 4, J], BF16, name="oh")
                    nc.vector.tensor_tensor(
                        out=oh[:],
                        in0=t2[:].unsqueeze(2).broadcast_to([128, 2, 4, J]),
                        in1=iotaj[:].unsqueeze(1).broadcast_to([128, 2, 4, J]),
                        op=OP.is_equal)
                    ub = u_both[:].unsqueeze(2).broadcast_to([128, 2, 4, J])
                    xb = x_both[:].unsqueeze(2).broadcast_to([128, 2, 4, J])
                    x2b = x2_both[:].unsqueeze(2).broadcast_to([128, 2, 4, J])
                    nc.vector.tensor_tensor(out=B[:, :, :, 0, :], in0=oh[:],
                                            in1=ub, op=OP.mult)
                    nc.vector.tensor_tensor(out=B[:, :, :, 1, :],
                                            in0=B[:, :, :, 0, :],
                                            in1=xb, op=OP.mult)
                    for k in range(2, 8):
                        nc.vector.tensor_tensor(out=B[:, :, :, k, :],
                                                in0=B[:, :, :, k - 1, :],
                                                in1=x2b, op=OP.mult)
                        nc.vector.tensor_tensor(out=B[:, :, :, k, :],
                                                in0=B[:, :, :, k, :],
                                                in1=B[:, :, :, k - 2, :],
                                                op=OP.subtract)
                    return B

                def pe_stage(B):
                    """8 batches of (8 transposes -> cin copy -> 8 matmuls ->
                    cnd copy).  Transpose input for pair-col c is the strided
                    AP [p, feat(64), jj(2)] -> loaded rows r = 2f+jj."""
                    # J = (jj, c): pair-column c holds J-cols c and c+64, so
                    # the 128 pair values sit at a single stride of 64:
                    # address = c + 64*(2f + jj)
                    Bj = B[:].rearrange("p h q k (jj c) -> p (h q k jj) c",
                                        jj=2)
                    cndb = cndpool.tile([128, 64, 128], BF16, name="cndb")

                    def mm_batch(bb, cin):
                        ps2 = ppool2.tile([128, 8, 128], F32, space="PSUM",
                                          name="ps2")
                        for jcol in range(8):
                            nc.tensor.matmul(out=ps2[:, jcol, :],
                                             lhsT=cin[:, jcol, :],
                                             rhs=w4[:], start=True, stop=True)
                        nc.scalar.activation(
                            cndb[:, 8 * bb:8 * bb + 8, :]
                            .rearrange("p c f -> p (c f)"),
                            ps2[:].rearrange("p c f -> p (c f)"), AF.Identity)

                    prev = None
                    for b in range(8):
                        ps1 = ppool1.tile([128, 8, 128], BF16, space="PSUM",
                                          name="ps1")
                        for jcol in range(8):
                            c = 8 * b + jcol
                            nc.tensor.transpose(
                                out=ps1[:, jcol, :],
                                in_=Bj[:, :, c],
                                identity=ident[:])
                        cin = cpool.tile([128, 8, 128], BF16, name="cin")
                        nc.scalar.activation(
                            cin[:].rearrange("p c f -> p (c f)"),
                            ps1[:].rearrange("p c f -> p (c f)"), AF.Identity)
                        if prev is not None:
                            mm_batch(prev[0], prev[1])
                        prev = (b, cin)
                    mm_batch(prev[0], prev[1])
                    return cndb

                def post_stage(cndb, fld, ct2):
                    """ti-select + product + P_l outer + SEG pre-reduce."""
                    oh_ti = wpool.tile([128, J, 4], BF16, name="oh_ti")
                    nc.vector.tensor_tensor(
                        out=oh_ti[:],
                        in0=fld["ti"][:].unsqueeze(2).broadcast_to([128, J, 4]),
                        in1=iotaj[:, :, 0].unsqueeze(1)
                            .broadcast_to([128, J, 4]),
                        op=OP.is_equal)
                    # cnd cols o = jj*64 + h*32 + d*4 + ti; row c of cndb is
                    # pair-col c = (J-cols c, c+64)  ->  field j = jj*64 + c
                    cv = cndb[:].rearrange("p c (jj h f) -> p jj c h f",
                                           jj=2, h=2)
                    sel0 = ppool.tile([128, 2, 64, 32], BF16, name="sel0")
                    ohv = oh_ti[:].rearrange("p (jj c) t -> p jj c t", jj=2)
                    for jj in range(2):
                        nc.vector.tensor_tensor(
                            out=sel0[:, jj].rearrange("p c (d t) -> p c d t", t=4),
                            in0=cv[:, jj, :, 0, :]
                                .rearrange("p c (d t) -> p c d t", t=4),
                            in1=ohv[:, jj].unsqueeze(2)
                                .broadcast_to([128, 64, 8, 4]),
                            op=OP.mult)
                    prod = ppool.tile([128, J, 32], BF16, name="prod")
                    prodv = prod[:].rearrange("p (jj c) f -> p jj c f", jj=2)
                    for jj in range(2):
                        nc.vector.tensor_tensor(
                            out=prodv[:, jj],
                            in0=sel0[:, jj],
                            in1=cv[:, jj, :, 1, :], op=OP.mult)
                    # sum over ti (innermost 4)
                    pv = prod[:].rearrange("p j (d a b) -> p j d a b", a=2, b=2)
                    a0 = ppool.tile([128, J, 8, 2], BF16, name="a0")
                    nc.vector.tensor_tensor(out=a0[:], in0=pv[:, :, :, 0, :],
                                            in1=pv[:, :, :, 1, :], op=OP.add)
                    h8 = ppool.tile([128, J, 8], BF16, name="h8")
                    nc.vector.tensor_tensor(out=h8[:], in0=a0[:, :, :, 0],
                                            in1=a0[:, :, :, 1], op=OP.add)
                    Pt, p3t = ct2
                    nc.vector.tensor_tensor(out=Pt[:, 2, :], in0=p3t[:],
                                            in1=Pt[:, 0, :], op=OP.mult)
                    # ang for l=1..3 [p, J, 3, 8]
                    ang = ppool.tile([128, J, 3, 8], BF16, name="ang")
                    nc.vector.tensor_tensor(
                        out=ang[:],
                        in0=Pt[:].rearrange("p l j -> p j l").unsqueeze(3)
                            .broadcast_to([128, J, 3, 8]),
                        in1=h8[:].unsqueeze(2).broadcast_to([128, J, 3, 8]),
                        op=OP.mult)
                    # SEG=16 pre-reduce by contiguous halves;
                    # partials [p, MB/128, (l,d)=32] f32, l=0 from h8.
                    partials = spool.tile([128, MB // 128, 32], F32,
                                          name="partials")
                    NB = MB // 128      # blocks per partition per macro (8)
                    hv = h8[:].rearrange("p (g s) d -> p g s d", g=NB)
                    t1 = ppool.tile([128, NB, 8, 8], BF16, name="t1")
                    nc.vector.tensor_tensor(out=t1[:], in0=hv[:, :, 0:8, :],
                                            in1=hv[:, :, 8:16, :], op=OP.add)
                    t2s = ppool.tile([128, NB, 4, 8], BF16, name="t2s")
                    nc.vector.tensor_tensor(out=t2s[:], in0=t1[:, :, 0:4, :],
                                            in1=t1[:, :, 4:8, :], op=OP.add)
                    t3 = ppool.tile([128, NB, 2, 8], BF16, name="t3")
                    nc.vector.tensor_tensor(out=t3[:], in0=t2s[:, :, 0:2, :],
                                            in1=t2s[:, :, 2:4, :], op=OP.add)
                    nc.vector.tensor_tensor(out=partials[:, :, 0:8],
                                            in0=t3[:, :, 0, :],
                                            in1=t3[:, :, 1, :], op=OP.add)
                    av = ang[:].rearrange("p (g s) l d -> p g s (l d)", g=NB)
                    u1 = ppool.tile([128, NB, 8, 24], BF16, name="u1")
                    nc.vector.tensor_tensor(out=u1[:], in0=av[:, :, 0:8, :],
                                            in1=av[:, :, 8:16, :], op=OP.add)
                    u2 = ppool.tile([128, NB, 4, 24], BF16, name="u2")
                    nc.vector.tensor_tensor(out=u2[:], in0=u1[:, :, 0:4, :],
                                            in1=u1[:, :, 4:8, :], op=OP.add)
                    u3 = ppool.tile([128, NB, 2, 24], BF16, name="u3")
                    nc.vector.tensor_tensor(out=u3[:], in0=u2[:, :, 0:2, :],
                                            in1=u2[:, :, 2:4, :], op=OP.add)
                    nc.vector.tensor_tensor(out=partials[:, :, 8:32],
                                            in0=u3[:, :, 0, :],
                                            in1=u3[:, :, 1, :], op=OP.add)
                    return partials

                ld = {0: dma_loads(0)}
                if nmacro > 1:
                    ld[1] = dma_loads(1)
                ap = {0: act_pre(ld[0][0])}
                F1cur = build_features(ld[0][0], ap[0][0], ap[0][1])
                for k in range(1, KACC):
                    qf = qacc[k].ap().rearrange("(p r) s -> p (r s)", p=128)
                    w = QPAD * STRIDE // 128
                    for i in range(0, w, 512):
                        nc.sync.dma_start(out=qf[:, i:i + 512],
                                          in_=zero[:, :min(512, w - i)])

                # reduce group g: emitted after macro mg[g] (one extra macro
                # of slack so its scatter drains are done before the load)
                red_at = {}
                for g in range(NGRP):
                    mm = min(mg[g] + 1, nmacro - 1) if g < NGRP - 1 else nmacro
                    red_at.setdefault(mm, []).append(g)

                for m in range(nmacro):
                    fld, idxs = ld[m]
                    u_both, s_both, ct2 = ap[m]
                    F1use = F1cur
                    if m + 2 < nmacro:
                        ld[m + 2] = dma_loads(m + 2)
                    if m + 1 < nmacro:
                        ap[m + 1] = act_pre(ld[m + 1][0])
                        F1cur = build_features(ld[m + 1][0], ap[m + 1][0],
                                               ap[m + 1][1])
                    cndb = pe_stage(F1use)
                    partials = post_stage(cndb, fld, ct2)
                    ld.pop(m, None)
                    ap.pop(m, None)

                    for (ci, off, pl) in by_macro[m]:
                        nc.gpsimd.dma_scatter_add(
                            qacc[ci % KACC].ap()[:, :DL],
                            partials[:, off // 128:(off + pl) // 128, :],
                            idxs[:, off // 16:(off + pl) // 16],
                            pl, pl, DL, elem_step=STRIDE,
                            queue_num=0)

                    for g in red_at.get(m, []):
                        reduce_group(g)

                for g in red_at.get(nmacro, []):
                    reduce_group(g)
    nc.compile()
    return nc


def _install_ntff_hook():
    """Provide antenv.axon_hooks (missing in this image) via sys.modules so
    run_bass_kernel_spmd(trace=True) can capture NTFF profiles."""
    import types, ctypes, contextlib
    try:
        from antenv.axon_hooks import get_axon_ntff_profile_hook  # noqa: F401
        return
    except ImportError:
        pass
    so_path = "/opt/axon/libaxon_pjrt.so"
    try:
        lib = ctypes.CDLL(so_path)
    except OSError:
        return
    if not hasattr(lib, "axon_start_nrt_profile"):
        return
    lib.axon_start_nrt_profile.argtypes = [ctypes.POINTER(ctypes.c_int64),
                                           ctypes.c_size_t]
    lib.axon_start_nrt_profile.restype = ctypes.c_int64
    lib.axon_stop_nrt_profile.argtypes = [ctypes.c_char_p]
    lib.axon_stop_nrt_profile.restype = ctypes.c_int64

    @contextlib.contextmanager
    def _hook(output_dir, device_ids):
        import jax
        jax.devices()
        if device_ids:
            ids = (ctypes.c_int64 * len(device_ids))(*device_ids)
            rc = lib.axon_start_nrt_profile(ids, len(device_ids))
        else:
            rc = lib.axon_start_nrt_profile(None, 0)
        if rc != 0:
            raise RuntimeError(f"axon_start_nrt_profile rc={rc}")
        try:
            yield
        finally:
            n = lib.axon_stop_nrt_profile(str(output_dir).encode())
            if n <= 0:
                print(f"ntff capture wrote {n} files", flush=True)

    mod = types.ModuleType("antenv.axon_hooks")
    mod.get_axon_ntff_profile_hook = lambda: _hook
    mod.set_axon_ntff_profile_hook = lambda h: None
    import antenv
    sys.modules["antenv.axon_hooks"] = mod
    antenv.axon_hooks = mod


_CACHE = {}


def kernel(n_atoms, triplet_index, r_ij, r_ik, cos_theta,
           type_i, type_j, type_k, c_table, _sim=False, _trace=False):
    cores, consts, chunks, nmacro, TPAD, mg = _host_prep(
        n_atoms, triplet_index, r_ij, r_ik, cos_theta,
        type_i, type_j, type_k, c_table)
    key = (nmacro, TPAD, mg, tuple(chunks))
    if key not in _CACHE:
        _CACHE[key] = _build(chunks, nmacro, TPAD, mg)
    nc = _CACHE[key]
    in_maps = []
    for c in range(N_CORES):
        m = dict(cores[c])
        m.update(consts)
        in_maps.append(m)
    if _sim:
        from concourse import bass_interp
        sim = bass_interp.MultiCoreSim(nc, N_CORES)
        for c in range(N_CORES):
            for k, v in in_maps[c].items():
                sim.cores[c].tensor(k)[:] = v
        sim.simulate()
        out = np.array(sim.cores[0].mem_tensor("out"))
    else:
        if _trace:
            _install_ntff_hook()
        last_err = None
        for _try in range(3):
            try:
                res = run_bass_kernel_spmd(nc, in_maps,
                                           core_ids=list(range(N_CORES)),
                                           trace=_trace)
                out = np.asarray(res.results[0]["out"])
                break
            except Exception as e:  # transient device-unrecoverable after a crash
                last_err = e
        else:
            raise last_err
        kernel.last_exec_ns = res.exec_time_ns
        kernel.last_results = res
    # device q rows are (a%128)*160 + a//128 and columns are (l, d);
    # un-permute and transpose to (a, d, l)
    qfull = out.reshape(128, QPAD // 128, DL)
    a = np.arange(N_ATOMS)
    q = qfull[a % 128, a // 128]
    return (q.reshape(N_ATOMS, L_MAX, N_DESC).transpose(0, 2, 1)
            .astype(np.float32))


# revision 16
# speedup vs baseline: 1.9391x; 1.0872x over previous
"""AngularDescriptor Trainium2 kernel (8 NeuronCores, SPMD + AllReduce).

Per core: T/8 triplets.  Device computes Chebyshev/Legendre bases, the
per-pair-type radial einsum (PE matmul with fixed block-diag weights after a
4-way tj/tk one-hot expansion; 4-way ti select on DVE), the outer product
ang = (g_ij*g_ik) (x) P_l, and segment-sums ang into q[20000,8,4] via
gpsimd.dma_scatter_add.  HW scatter-add loses duplicate indices within one
instruction (last-write-wins race), so the host orders each shard's blocks
into occurrence-rank classes (class r = r-th block of an atom on this core):
within a class all atom indices are unique.  Blocks are SEG=16 same-atom
triplet groups formed on the GLOBAL atom-sorted order and dealt round-robin
to cores, which keeps SEG padding global and class sizes balanced.  Classes
are cut into chunks that rotate over KACC DRAM accumulators, so
same-accumulator scatters serialize (WAW dep) while different-accumulator
scatters overlap.  Padding slots scatter to distinct dummy atom rows
(20000..20479) so one uniform program serves all cores.

The q rows are split into NGRP=4 groups of 32 q-partitions; each group's
K-way add + AllReduce + output DMA runs mid-stream as soon as its scatters
complete, so only the last (quarter-sized) group's reduce sits on the tail.

Layout/pipeline notes:
 - The tj/tk one-hot is FUSED into the Chebyshev recurrence: the recurrence
   state B[h,q,k] = onehot(q)*T_k(x_h)*u_h runs in layout [p, h, q, k, J]
   (J innermost => all DVE ops are long-run stride-1 bf16, 2x mode).  The
   PE transpose reads pair-column c via the strided AP [p, feat(64), jj(2)]
   so the (feat,jj) row interleave costs nothing on DVE.
 - W output columns are (jj, h, ti, d) with d innermost; the ti-select adds
   are contiguous 8-wide slices.  q column order is (l,d); the host
   transposes to (d,l) for free.
 - Legendre P is l-major [p, l, J]; P_0==1 is never materialized (the l=0
   partials pre-reduce h itself).  Pre-reduce trees pair contiguous halves
   so every level keeps >=32-element runs.
 - Per macro the PE work is 8 batches of (8 transposes -> 1 ACT cin copy ->
   8 matmuls -> 1 ACT cnd copy); DVE emits next macro's feature build before
   this macro's post-processing so it never waits on PE/ACT.
"""
import sys

sys.path.insert(0, "/opt/trn_rl_repo")
import numpy as np

from concourse import bass, bacc, mybir, tile
from concourse.bass_utils import run_bass_kernel_spmd

N_TYPES, N_DESC, K_MAX, L_MAX = 4, 8, 8, 4
R_C = 5.0
N_ATOMS = 20000
N_CORES = 8
DL = N_DESC * L_MAX          # 32
QPAD = 20480                 # 128 * 160
STRIDE = 64                  # q row stride in f32 (256B; scatter needs 256B mult)
J = 128                      # field columns per macro-tile
MACRO = 128 * J              # 16384 triplets per macro
KACC = 2                     # rotating DRAM accumulators
SEG = 16                     # triplets pre-reduced per scattered block
MB = MACRO // SEG            # blocks per macro (1024)
CHUNK = 1024                 # max idxs (blocks) per scatter instruction
NGRP = 4                     # staged reduce groups (q-row ranges)
GROW_BOUNDS = (48, 96, 136, 160)   # cumulative q-row bounds per group
F32, BF16, I16 = mybir.dt.float32, mybir.dt.bfloat16, mybir.dt.int16
PI = float(np.pi)


def _host_prep(n_atoms, triplet_index, r_ij, r_ik, cos_theta,
               type_i, type_j, type_k, c_table):
    """Global atom-sort -> SEG-blocks -> deal blocks round-robin to cores ->
    per-core occurrence-rank classes (per reduce group) -> uniform layout.

    Block b of a core lives at partition b%128, columns SEG*(b//128)..+SEG-1
    (block-major columns).  Pad slots use r=r_c so fc=0 => ang=0."""
    import ml_dtypes
    T = triplet_index.shape[0]
    atom_all = np.asarray(triplet_index[:, 0], dtype=np.int64)

    # ---- global blocks ----
    order = np.argsort(atom_all, kind="stable")
    sa = atom_all[order]
    first = np.r_[True, sa[1:] != sa[:-1]]
    idxf = np.where(first)[0]
    counts = np.diff(np.r_[idxf, T])
    uatoms = sa[idxf]
    nblk_per_atom = -(-counts // SEG)
    nblk_tot = int(nblk_per_atom.sum())

    blk_atom = np.repeat(uatoms, nblk_per_atom)
    starts = np.r_[0, np.cumsum(counts)[:-1]]
    blk_rank_g = (np.arange(nblk_tot)
                  - np.repeat(np.r_[0, np.cumsum(nblk_per_atom)[:-1]],
                              nblk_per_atom))
    blk_start = np.repeat(starts, nblk_per_atom) + blk_rank_g * SEG
    blk_cnt = np.minimum(
        np.repeat(counts, nblk_per_atom) - blk_rank_g * SEG, SEG)

    # ---- deal blocks to cores (rotating offset per atom) ----
    core_of = (blk_rank_g + np.repeat(uatoms, nblk_per_atom)) % N_CORES
    rank_c = blk_rank_g // N_CORES

    # reduce groups by q-row ranges (48,48,40,24 of 160): the last (tail)
    # group is smallest so the final reduce+AllReduce is short
    gb = np.array(GROW_BOUNDS[:-1]) * 128      # atom-id bounds
    grp = np.searchsorted(gb, blk_atom, side="right")

    # ---- class sizes (uniform across cores) ----
    maxr = int(rank_c.max()) + 1
    cls_cnt = np.zeros((N_CORES, NGRP, maxr), dtype=np.int64)
    np.add.at(cls_cnt, (core_of, grp, rank_c), 1)
    cls_list = []                        # [padded_blk_count, g, r]; fillers g=None
    o = 0
    g_end = []
    for g in range(NGRP):
        for r in range(maxr):
            mx = int(cls_cnt[:, g, r].max())
            if mx == 0:
                continue
            p = -(-mx // 128) * 128
            cls_list.append([p, g, r])
            o += p
        g_end.append(o)
    fill = (-o) % MB
    if fill:
        cls_list.append([fill, None, None])
        o += fill
    TBLK = o
    nmacro = TBLK // MB
    TPAD = TBLK * SEG
    # reduce group g is complete after macro mg[g]-1 (all its chunks before)
    mg = [min(-(-e // MB), nmacro) for e in g_end]

    # chunk table in blocks: (start_blk, len_blk); chunks never cross class
    # or macro boundaries; filler ranges are never scattered
    chunks = []
    o = 0
    for p, g, r in cls_list:
        if g is not None:
            sblk = 0
            while sblk < p:
                cl = min(CHUNK, p - sblk)
                mstart = (o + sblk) // MB
                if (o + sblk + cl - 1) // MB != mstart:
                    cl = (mstart + 1) * MB - (o + sblk)
                chunks.append((o + sblk, cl))
                sblk += cl
        o += p

    # ---- destination block id per global block ----
    cls_off = {}
    o = 0
    for p, g, r in cls_list:
        if g is not None:
            cls_off[(g, r)] = o
        o += p
    fields_src = dict(r_ij=np.asarray(r_ij, np.float32),
                      r_ik=np.asarray(r_ik, np.float32),
                      ct=np.asarray(cos_theta, np.float32),
                      ti=np.asarray(type_i, np.float32),
                      tj=np.asarray(type_j, np.float32),
                      tk=np.asarray(type_k, np.float32))
    G = TPAD // 128
    cores = []
    for c in range(N_CORES):
        m = core_of == c
        b_atom = blk_atom[m]
        b_g = grp[m]
        b_r = rank_c[m]
        b_start = blk_start[m]
        b_cnt = blk_cnt[m]
        key = b_g * maxr + b_r
        ordk = np.lexsort((np.arange(len(key)), key))
        pos = np.empty(len(key), dtype=np.int64)
        kk = key[ordk]
        kfirst = np.r_[True, kk[1:] != kk[:-1]]
        kidx = np.where(kfirst)[0]
        within = np.arange(len(key)) - np.repeat(kidx, np.diff(np.r_[kidx, len(key)]))
        pos[ordk] = within
        dst_blk = np.array([cls_off[(g, r)] for g, r in zip(b_g, b_r)],
                           dtype=np.int64) + pos

        dev = {}
        for n in fields_src:
            fillv = R_C if n in ("r_ij", "r_ik") else 0.0
            dt = ml_dtypes.bfloat16 if n in ("ti", "tj", "tk") else np.float32
            dev[n] = np.full((128, G), fillv, dtype=dt)
        # q row for atom a is (a%128)*160 + a//128: each reduce group's
        # atoms [5120g, 5120(g+1)) form row range [40g, 40g+40) on ALL 128
        # partitions, so the staged collectives run full-width.
        bidx = np.empty(TBLK, dtype=np.int16)
        dum = 20000 + (np.arange(TBLK, dtype=np.int64) % 480)
        bidx[:] = ((dum % 128) * 160 + dum // 128).astype(np.int16)
        bidx[dst_blk] = ((b_atom % 128) * 160 + b_atom // 128).astype(np.int16)

        slot_b = np.repeat(dst_blk, b_cnt)
        slot_s = (np.arange(int(b_cnt.sum()))
                  - np.repeat(np.r_[0, np.cumsum(b_cnt)[:-1]], b_cnt))
        src_idx = order[np.repeat(b_start, b_cnt) + slot_s]
        dst_p = slot_b % 128
        dst_c = SEG * (slot_b // 128) + slot_s
        for n in dev:
            dev[n][dst_p, dst_c] = fields_src[n][src_idx]
        arrays = {n: dev[n] for n in dev}
        arrays["idx"] = np.tile(bidx.reshape(TBLK // 16, 16).T, (8, 1)).copy()
        cores.append(arrays)

    # ---- weight table ----
    # basis fold: reference uses (T_k + 1)*u; we feed T_k*u:
    #   c'[d,0] += sum_k c[d,k]
    ctab = np.asarray(c_table, dtype=np.float64).copy()
    ctab[:, :, :, 0] += ctab.sum(axis=3)
    ctab = ctab.astype(np.float32)
    # rows r = 2*f + jj with f = (h,q,k) = h*32+q*8+k; jj in {0,1} selects
    # the pair member (transpose column c covers J-cols c and c+64)
    # cols o = jj*64 + h*32 + d*4 + ti   (ti innermost)
    W4p = np.zeros((128, 128), dtype=np.float32)
    for h in range(2):
        for q in range(4):
            for k in range(8):
                f = h * 32 + q * 8 + k
                for jj in range(2):
                    for d in range(8):
                        for ti in range(4):
                            W4p[2 * f + jj,
                                jj * 64 + h * 32 + d * 4 + ti] = ctab[ti, q, d, k]
    iotaJ = np.tile(np.arange(4, dtype=np.float32)[:, None], (1, J))
    iotaJ = np.tile(iotaJ.reshape(1, 4 * J), (128, 1))   # [128, 4*J] value=q
    consts = dict(w4=W4p, ident=np.eye(128, dtype=np.float32), iotaj=iotaJ)
    return cores, consts, chunks, nmacro, TPAD, tuple(mg)


def _build(chunks, nmacro, TPAD, mg):
    G = TPAD // 128
    nc = bacc.Bacc(None, target_bir_lowering=False, num_devices=N_CORES,
                   dynamic_dma_scratch_size=32768, num_swdge_queues=1)
    P = {}
    for n in ("r_ij", "r_ik", "ct", "ti", "tj", "tk"):
        fdt = BF16 if n in ("ti", "tj", "tk") else F32
        P[n] = nc.declare_dram_parameter(n, [128, G], fdt, isOutput=False)
    P["idx"] = nc.declare_dram_parameter("idx", [128, TPAD // SEG // 16], I16,
                                         isOutput=False)
    P["w4"] = nc.declare_dram_parameter("w4", [128, 128], F32, isOutput=False)
    P["ident"] = nc.declare_dram_parameter("ident", [128, 128], F32, isOutput=False)
    P["iotaj"] = nc.declare_dram_parameter("iotaj", [128, 4 * J], F32,
                                           isOutput=False)
    out_d = nc.declare_dram_parameter("out", [128, QPAD * DL // 128], F32,
                                      isOutput=True)

    qacc = [nc.dram_tensor(f"qacc{k}", [QPAD, STRIDE], F32) for k in range(KACC)]
    grows = [GROW_BOUNDS[0]] + [GROW_BOUNDS[g] - GROW_BOUNDS[g - 1]
                                for g in range(1, NGRP)]
    bnc_in = [nc.dram_tensor(f"bounce_in{g}", [128, grows[g] * DL], F32)
              for g in range(NGRP)]
    bnc_out = [nc.dram_tensor(f"bounce_out{g}", [128, grows[g] * DL], F32,
                              addr_space="Shared") for g in range(NGRP)]

    AF = mybir.ActivationFunctionType
    OP = mybir.AluOpType

    with tile.TileContext(nc) as tc:
        with tc.tile_pool(name="const", bufs=1) as cst:
            w4 = cst.tile([128, 128], BF16)
            ident = cst.tile([128, 128], BF16)
            iotaj = cst.tile([128, 4, J], BF16)
            tmpf = cst.tile([128, 4 * J], F32)
            zero = cst.tile([128, 512], F32)
            halfpi = cst.tile([128, 1], F32)
            nc.vector.memset(halfpi[:], PI / 2)
            negone = cst.tile([128, 1], F32)
            nc.vector.memset(negone[:], -1.0)
            neghalf = cst.tile([128, 1], F32)
            nc.vector.memset(neghalf[:], -0.5)
            neg32 = cst.tile([128, 1], F32)
            nc.vector.memset(neg32[:], -1.5)
            negtwo = cst.tile([128, 1], F32)
            nc.vector.memset(negtwo[:], -2.0)
            nc.sync.dma_start(out=tmpf[:, :128], in_=P["w4"][:])
            nc.vector.tensor_copy(out=w4[:], in_=tmpf[:, :128])
            nc.sync.dma_start(out=tmpf[:, :128], in_=P["ident"][:])
            nc.vector.tensor_copy(out=ident[:], in_=tmpf[:, :128])
            nc.sync.dma_start(out=tmpf[:], in_=P["iotaj"][:])
            nc.vector.tensor_copy(
                out=iotaj[:].rearrange("p q j -> p (q j)"), in_=tmpf[:])
            nc.vector.memset(zero[:], 0.0)

            with (
                tc.tile_pool(name="fields", bufs=4) as fpool,
                tc.tile_pool(name="idxp", bufs=4) as ipool,
                tc.tile_pool(name="work", bufs=2) as wpool,
                tc.tile_pool(name="f1p", bufs=2) as f1pool,
                tc.tile_pool(name="cinp", bufs=3) as cpool,
                tc.tile_pool(name="cndp", bufs=2) as cndpool,
                tc.tile_pool(name="postp", bufs=1) as ppool,
                tc.tile_pool(name="redp", bufs=2) as redp,
                tc.tile_pool(name="scat", bufs=3) as spool,
                tc.tile_pool(name="ps1", bufs=2, space="PSUM") as ppool1,
                tc.tile_pool(name="ps2", bufs=1, space="PSUM") as ppool2,
            ):
                by_macro = [[] for _ in range(nmacro)]
                for ci, (s, pl) in enumerate(chunks):
                    by_macro[s // MB].append((ci, s % MB, pl))
                NIC = MB // 16   # idx cols per macro

                NR = QPAD // 128
                qv32 = [q.ap().rearrange("(p r) s -> p r s", p=128)[:, :, :DL]
                        for q in qacc]

                def reduce_group(g):
                    """K-way add + AllReduce + output DMA for q rows
                    [g*NR/NGRP, (g+1)*NR/NGRP) on all 128 partitions."""
                    r0 = 0 if g == 0 else GROW_BOUNDS[g - 1]
                    r1 = GROW_BOUNDS[g]
                    acc = redp.tile([128, 48, DL], F32,
                                    name="acc")[:, :r1 - r0]
                    nc.sync.dma_start(out=acc[:], in_=qv32[0][:, r0:r1])
                    for k in range(1, KACC):
                        nc.gpsimd.dma_start(out=acc[:], in_=qv32[k][:, r0:r1],
                                            accum_op=OP.add)
                    c0, c1 = r0 * DL, r1 * DL
                    nc.sync.dma_start(out=bnc_in[g].ap()[:],
                                      in_=acc[:]
                                      .rearrange("p r s -> p (r s)"))
                    nc.gpsimd.collective_compute(
                        "AllReduce", OP.add,
                        replica_groups=[list(range(N_CORES))],
                        ins=[bnc_in[g].ap()[:].opt()],
                        outs=[bnc_out[g].ap()[:].opt()])
                    nc.sync.dma_start(out=out_d.ap()[:, c0:c1],
                                      in_=bnc_out[g].ap()[:])

                def dma_loads(m):
                    fld = {}
                    for n in ("r_ij", "r_ik", "ct", "ti"):
                        fdt = BF16 if n == "ti" else F32
                        t = fpool.tile([128, J], fdt, name=f"fld_{n}")
                        nc.sync.dma_start(out=t[:], in_=P[n][:, m * J:(m + 1) * J])
                        fld[n] = t
                    t2 = fpool.tile([128, 2, J], BF16, name="fld_t2")
                    nc.sync.dma_start(out=t2[:, 0, :],
                                      in_=P["tj"][:, m * J:(m + 1) * J])
                    nc.sync.dma_start(out=t2[:, 1, :],
                                      in_=P["tk"][:, m * J:(m + 1) * J])
                    fld["t2"] = t2
                    idxs = ipool.tile([128, NIC], I16, name="idxs")
                    nc.sync.dma_start(out=idxs[:],
                                      in_=P["idx"][:, m * NIC:(m + 1) * NIC])
                    return fld, idxs

                def act_pre(fld):
                    """u = 0.5*sin^2(pi/2 - pi*r/(2rc)); s = (r/rc - 1)^2."""
                    u_both = wpool.tile([128, 2, J], BF16, name="u_both")
                    s_both = wpool.tile([128, 2, J], F32, name="s_both")
                    for half, rn in enumerate(("r_ij", "r_ik")):
                        r = fld[rn]
                        utmp = wpool.tile([128, J], F32, name=f"utmp{half}")
                        nc.scalar.activation(utmp[:], r[:], AF.Sin,
                                             bias=halfpi[:], scale=-PI / (2 * R_C))
                        nc.scalar.activation(u_both[:, half, :], utmp[:], AF.Square,
                                             scale=float(np.sqrt(0.5)))
                        nc.scalar.activation(s_both[:, half, :], r[:], AF.Square,
                                             bias=negone[:], scale=1.0 / R_C)
                    # Legendre P l-major [p, 3, J]: P1=ct, P2=1.5ct2-.5,
                    # p3t=2.5ct2-1.5 (P3 = p3t*ct on DVE later)
                    x_both = wpool.tile([128, 2, J], BF16, name="x_both")
                    x2_both = wpool.tile([128, 2, J], BF16, name="x2_both")
                    nc.scalar.activation(x_both[:].rearrange("p h j -> p (h j)"),
                                         s_both[:].rearrange("p h j -> p (h j)"),
                                         AF.Identity, bias=negone[:], scale=2.0)
                    nc.scalar.activation(x2_both[:].rearrange("p h j -> p (h j)"),
                                         s_both[:].rearrange("p h j -> p (h j)"),
                                         AF.Identity, bias=negtwo[:], scale=4.0)
                    ct2 = wpool.tile([128, J], F32, name="ct2")
                    nc.scalar.activation(ct2[:], fld["ct"][:], AF.Square)
                    Pt = wpool.tile([128, 3, J], BF16, name="Pt")
                    nc.scalar.activation(Pt[:, 0, :], fld["ct"][:], AF.Identity)
                    nc.scalar.activation(Pt[:, 1, :], ct2[:], AF.Identity,
                                         bias=neghalf[:], scale=1.5)
                    p3t = wpool.tile([128, J], BF16, name="p3t")
                    nc.scalar.activation(p3t[:], ct2[:], AF.Identity,
                                         bias=neg32[:], scale=2.5)
                    return u_both, (s_both, x_both, x2_both), (Pt, p3t)

                def build_features(fld, u_both, s_both):
                    """B[h,q,k] = oh(q)*T_k(x_h)*u_h, layout [p, h, q, k, J]."""
                    B = f1pool.tile([128, 2, 4, 8, J], BF16, name="B")
                    x_both, x2_both = s_both[1], s_both[2]
                    t2 = fld["t2"]
                    oh = wpool.tile([128, 2, 4, J], BF16, name="oh")
                    nc.vector.tensor_tensor(
                        out=oh[:],
                        in0=t2[:].unsqueeze(2).broadcast_to([128, 2, 4, J]),
                        in1=iotaj[:].unsqueeze(1).broadcast_to([128, 2, 4, J]),
                        op=OP.is_equal)
                    ub = u_both[:].unsqueeze(2).broadcast_to([128, 2, 4, J])
                    xb = x_both[:].unsqueeze(2).broadcast_to([128, 2, 4, J])
                    x2b = x2_both[:].unsqueeze(2).broadcast_to([128, 2, 4, J])
                    nc.vector.tensor_tensor(out=B[:, :, :, 0, :], in0=oh[:],
                                            in1=ub, op=OP.mult)
                    nc.vector.tensor_tensor(out=B[:, :, :, 1, :],
                                            in0=B[:, :, :, 0, :],
                                            in1=xb, op=OP.mult)
                    for k in range(2, 8):
                        nc.vector.tensor_tensor(out=B[:, :, :, k, :],
                                                in0=B[:, :, :, k - 1, :],
                                                in1=x2b, op=OP.mult)
                        nc.vector.tensor_tensor(out=B[:, :, :, k, :],
                                                in0=B[:, :, :, k, :],
                                                in1=B[:, :, :, k - 2, :],
                                                op=OP.subtract)
                    return B

                def pe_stage(B):
                    """8 batches of (8 transposes -> cin copy -> 8 matmuls ->
                    cnd copy).  Transpose input for pair-col c is the strided
                    AP [p, feat(64), jj(2)] -> loaded rows r = 2f+jj."""
                    # J = (jj, c): pair-column c holds J-cols c and c+64, so
                    # the 128 pair values sit at a single stride of 64:
                    # address = c + 64*(2f + jj)
                    Bj = B[:].rearrange("p h q k (jj c) -> p (h q k jj) c",
                                        jj=2)
                    cndb = cndpool.tile([128, 64, 128], BF16, name="cndb")

                    def mm_batch(bb, cin):
                        ps2 = ppool2.tile([128, 16, 128], F32, space="PSUM",
                                          name="ps2")
                        for jcol in range(16):
                            nc.tensor.matmul(out=ps2[:, jcol, :],
                                             lhsT=cin[:, jcol, :],
                                             rhs=w4[:], start=True, stop=True)
                        nc.scalar.activation(
                            cndb[:, 16 * bb:16 * bb + 16, :]
                            .rearrange("p c f -> p (c f)"),
                            ps2[:].rearrange("p c f -> p (c f)"), AF.Identity)

                    prev = None
                    for b in range(4):
                        ps1 = ppool1.tile([128, 16, 128], BF16, space="PSUM",
                                          name="ps1")
                        for jcol in range(16):
                            c = 16 * b + jcol
                            nc.tensor.transpose(
                                out=ps1[:, jcol, :],
                                in_=Bj[:, :, c],
                                identity=ident[:])
                        cin = cpool.tile([128, 16, 128], BF16, name="cin")
                        nc.scalar.activation(
                            cin[:].rearrange("p c f -> p (c f)"),
                            ps1[:].rearrange("p c f -> p (c f)"), AF.Identity)
                        if prev is not None:
                            mm_batch(prev[0], prev[1])
                        prev = (b, cin)
                    mm_batch(prev[0], prev[1])
                    return cndb

                def post_stage(cndb, fld, ct2):
                    """ti-select + product + P_l outer + SEG pre-reduce."""
                    oh_ti = wpool.tile([128, J, 4], BF16, name="oh_ti")
                    nc.vector.tensor_tensor(
                        out=oh_ti[:],
                        in0=fld["ti"][:].unsqueeze(2).broadcast_to([128, J, 4]),
                        in1=iotaj[:, :, 0].unsqueeze(1)
                            .broadcast_to([128, J, 4]),
                        op=OP.is_equal)
                    # cnd cols o = jj*64 + h*32 + d*4 + ti; row c of cndb is
                    # pair-col c = (J-cols c, c+64)  ->  field j = jj*64 + c
                    cv = cndb[:].rearrange("p c (jj h f) -> p jj c h f",
                                           jj=2, h=2)
                    sel0 = ppool.tile([128, 2, 64, 32], BF16, name="sel0")
                    ohv = oh_ti[:].rearrange("p (jj c) t -> p jj c t", jj=2)
                    for jj in range(2):
                        nc.vector.tensor_tensor(
                            out=sel0[:, jj].rearrange("p c (d t) -> p c d t", t=4),
                            in0=cv[:, jj, :, 0, :]
                                .rearrange("p c (d t) -> p c d t", t=4),
                            in1=ohv[:, jj].unsqueeze(2)
                                .broadcast_to([128, 64, 8, 4]),
                            op=OP.mult)
                    prod = ppool.tile([128, J, 32], BF16, name="prod")
                    prodv = prod[:].rearrange("p (jj c) f -> p jj c f", jj=2)
                    for jj in range(2):
                        nc.vector.tensor_tensor(
                            out=prodv[:, jj],
                            in0=sel0[:, jj],
                            in1=cv[:, jj, :, 1, :], op=OP.mult)
                    # sum over ti (innermost 4)
                    pv = prod[:].rearrange("p j (d a b) -> p j d a b", a=2, b=2)
                    a0 = ppool.tile([128, J, 8, 2], BF16, name="a0")
                    nc.vector.tensor_tensor(out=a0[:], in0=pv[:, :, :, 0, :],
                                            in1=pv[:, :, :, 1, :], op=OP.add)
                    h8 = ppool.tile([128, J, 8], BF16, name="h8")
                    nc.vector.tensor_tensor(out=h8[:], in0=a0[:, :, :, 0],
                                            in1=a0[:, :, :, 1], op=OP.add)
                    Pt, p3t = ct2
                    nc.vector.tensor_tensor(out=Pt[:, 2, :], in0=p3t[:],
                                            in1=Pt[:, 0, :], op=OP.mult)
                    # ang for l=1..3 [p, J, 3, 8]
                    ang = ppool.tile([128, J, 3, 8], BF16, name="ang")
                    nc.vector.tensor_tensor(
                        out=ang[:],
                        in0=Pt[:].rearrange("p l j -> p j l").unsqueeze(3)
                            .broadcast_to([128, J, 3, 8]),
                        in1=h8[:].unsqueeze(2).broadcast_to([128, J, 3, 8]),
                        op=OP.mult)
                    # SEG=16 pre-reduce by contiguous halves;
                    # partials [p, MB/128, (l,d)=32] f32, l=0 from h8.
                    partials = spool.tile([128, MB // 128, 32], F32,
                                          name="partials")
                    NB = MB // 128      # blocks per partition per macro (8)
                    hv = h8[:].rearrange("p (g s) d -> p g s d", g=NB)
                    t1 = ppool.tile([128, NB, 8, 8], BF16, name="t1")
                    nc.vector.tensor_tensor(out=t1[:], in0=hv[:, :, 0:8, :],
                                            in1=hv[:, :, 8:16, :], op=OP.add)
                    t2s = ppool.tile([128, NB, 4, 8], BF16, name="t2s")
                    nc.vector.tensor_tensor(out=t2s[:], in0=t1[:, :, 0:4, :],
                                            in1=t1[:, :, 4:8, :], op=OP.add)
                    t3 = ppool.tile([128, NB, 2, 8], BF16, name="t3")
                    nc.vector.tensor_tensor(out=t3[:], in0=t2s[:, :, 0:2, :],
                                            in1=t2s[:, :, 2:4, :], op=OP.add)
                    nc.vector.tensor_tensor(out=partials[:, :, 0:8],
                                            in0=t3[:, :, 0, :],
                                            in1=t3[:, :, 1, :], op=OP.add)
                    av = ang[:].rearrange("p (g s) l d -> p g s (l d)", g=NB)
                    u1 = ppool.tile([128, NB, 8, 24], BF16, name="u1")
                    nc.vector.tensor_tensor(out=u1[:], in0=av[:, :, 0:8, :],
                                            in1=av[:, :, 8:16, :], op=OP.add)
                    u2 = ppool.tile([128, NB, 4, 24], BF16, name="u2")
                    nc.vector.tensor_tensor(out=u2[:], in0=u1[:, :, 0:4, :],
                                            in1=u1[:, :, 4:8, :], op=OP.add)
                    u3 = ppool.tile([128, NB, 2, 24], BF16, name="u3")
                    nc.vector.tensor_tensor(out=u3[:], in0=u2[:, :, 0:2, :],
                                            in1=u2[:, :, 2:4, :], op=OP.add)
                    nc.vector.tensor_tensor(out=partials[:, :, 8:32],
                                            in0=u3[:, :, 0, :],
                                            in1=u3[:, :, 1, :], op=OP.add)
                    return partials

                ld = {0: dma_loads(0)}
                if nmacro > 1:
                    ld[1] = dma_loads(1)
                ap = {0: act_pre(ld[0][0])}
                F1cur = build_features(ld[0][0], ap[0][0], ap[0][1])
                for k in range(KACC):
                    qf = qacc[k].ap().rearrange("(p r) s -> p (r s)", p=128)
                    w = QPAD * STRIDE // 128
                    for i in range(0, w, 512):
                        nc.sync.dma_start(out=qf[:, i:i + 512],
                                          in_=zero[:, :min(512, w - i)])

                # reduce group g: emitted after macro mg[g] (one extra macro
                # of slack so its scatter drains are done before the load)
                red_at = {}
                for g in range(NGRP):
                    mm = min(mg[g] + 1, nmacro - 1) if g < NGRP - 1 else nmacro
                    red_at.setdefault(mm, []).append(g)

                for m in range(nmacro):
                    fld, idxs = ld[m]
                    u_both, s_both, ct2 = ap[m]
                    F1use = F1cur
                    if m + 2 < nmacro:
                        ld[m + 2] = dma_loads(m + 2)
                    if m + 1 < nmacro:
                        ap[m + 1] = act_pre(ld[m + 1][0])
                        F1cur = build_features(ld[m + 1][0], ap[m + 1][0],
                                               ap[m + 1][1])
                    cndb = pe_stage(F1use)
                    partials = post_stage(cndb, fld, ct2)
                    ld.pop(m, None)
                    ap.pop(m, None)

                    for (ci, off, pl) in by_macro[m]:
                        nc.gpsimd.dma_scatter_add(
                            qacc[ci % KACC].ap()[:, :DL],
                            partials[:, off // 128:(off + pl) // 128, :],
                            idxs[:, off // 16:(off + pl) // 16],
                            pl, pl, DL, elem_step=STRIDE,
                            queue_num=0)

                    for g in red_at.get(m, []):
                        reduce_group(g)

                for g in red_at.get(nmacro, []):
                    reduce_group(g)
    nc.compile()
    return nc


def _install_ntff_hook():
    """Provide antenv.axon_hooks (missing in this image) via sys.modules so
    run_bass_kernel_spmd(trace=True) can capture NTFF profiles."""
    import types, ctypes, contextlib
    try:
        from antenv.axon_hooks import get_axon_ntff_profile_hook  # noqa: F401
        return
    except ImportError:
        pass
    so_path = "/opt/axon/libaxon_pjrt.so"
    try:
        lib = ctypes.CDLL(so_path)
    except OSError:
        return
    if not hasattr(lib, "axon_start_nrt_profile"):
        return
    lib.axon_start_nrt_profile.argtypes = [ctypes.POINTER(ctypes.c_int64),
                                           ctypes.c_size_t]
    lib.axon_start_nrt_profile.restype = ctypes.c_int64
    lib.axon_stop_nrt_profile.argtypes = [ctypes.c_char_p]
    lib.axon_stop_nrt_profile.restype = ctypes.c_int64

    @contextlib.contextmanager
    def _hook(output_dir, device_ids):
        import jax
        jax.devices()
        if device_ids:
            ids = (ctypes.c_int64 * len(device_ids))(*device_ids)
            rc = lib.axon_start_nrt_profile(ids, len(device_ids))
        else:
            rc = lib.axon_start_nrt_profile(None, 0)
        if rc != 0:
            raise RuntimeError(f"axon_start_nrt_profile rc={rc}")
        try:
            yield
        finally:
            n = lib.axon_stop_nrt_profile(str(output_dir).encode())
            if n <= 0:
                print(f"ntff capture wrote {n} files", flush=True)

    mod = types.ModuleType("antenv.axon_hooks")
    mod.get_axon_ntff_profile_hook = lambda: _hook
    mod.set_axon_ntff_profile_hook = lambda h: None
    import antenv
    sys.modules["antenv.axon_hooks"] = mod
    antenv.axon_hooks = mod


_CACHE = {}


def kernel(n_atoms, triplet_index, r_ij, r_ik, cos_theta,
           type_i, type_j, type_k, c_table, _sim=False, _trace=False):
    cores, consts, chunks, nmacro, TPAD, mg = _host_prep(
        n_atoms, triplet_index, r_ij, r_ik, cos_theta,
        type_i, type_j, type_k, c_table)
    key = (nmacro, TPAD, mg, tuple(chunks))
    if key not in _CACHE:
        _CACHE[key] = _build(chunks, nmacro, TPAD, mg)
    nc = _CACHE[key]
    in_maps = []
    for c in range(N_CORES):
        m = dict(cores[c])
        m.update(consts)
        in_maps.append(m)
    if _sim:
        from concourse import bass_interp
        sim = bass_interp.MultiCoreSim(nc, N_CORES)
        for c in range(N_CORES):
            for k, v in in_maps[c].items():
                sim.cores[c].tensor(k)[:] = v
        sim.simulate()
        out = np.array(sim.cores[0].mem_tensor("out"))
    else:
        if _trace:
            _install_ntff_hook()
        last_err = None
        for _try in range(3):
            try:
                res = run_bass_kernel_spmd(nc, in_maps,
                                           core_ids=list(range(N_CORES)),
                                           trace=_trace)
                out = np.asarray(res.results[0]["out"])
                break
            except Exception as e:  # transient device-unrecoverable after a crash
                last_err = e
        else:
            raise last_err
        kernel.last_exec_ns = res.exec_time_ns
        kernel.last_results = res
    # device q rows are (a%128)*160 + a//128 and columns are (l, d);
    # un-permute and transpose to (a, d, l)
    qfull = out.reshape(128, QPAD // 128, DL)
    a = np.arange(N_ATOMS)
    q = qfull[a % 128, a // 128]
    return (q.reshape(N_ATOMS, L_MAX, N_DESC).transpose(0, 2, 1)
            .astype(np.float32))
